# revision 3
# baseline (speedup 1.0000x reference)
"""FMoE forward (NaiveGate top-2, per-expert FFN, score-weighted combine) on 8 trn2 cores.

Strategy: tensor-parallel over D_HIDDEN. Each core holds a 512-wide hidden
slice of ALL 8 experts' W1/W2 (16MB resident in SBUF, same as one full
expert) and processes ALL 8192 dispatched token-expert pairs against its
slice -- perfect load balance by construction, no capacity padding to the
max expert load. The host computes the gate, orders pairs by expert
(segment structure identical on every core, so the kernel stays SPMD),
and sums the 8 partial Y outputs (each a 512-contraction slice of mm2).

Device kernel (per core, fp16 matmuls, fp32 accum):
  mm1: stationary = W1[e] slice chunk [128k, 128h], moving = X^T slab
       [128k, S] -> H^T chunk [128h, S] PSUM (8 k-chunks); ScalarE
       tanh-gelu (+b1), VectorE multiply by per-pair gate weight (fp16).
  mm2: stationary = W2[e] slice chunk [128h, 128d], moving = H^T chunk
       -> Y^T [128d, S] PSUM (4 h-chunks), fp16 copy out (scalar/vector
       alternating), DMA out.
Software pipeline: mm2 of slab s is emitted after mm1 of slab s+1 so the
PE never stalls on the last gelu of a slab. Weight/X/gate DMAs are issued
from separate engine sequencers (descriptor gen is ~0.7us per dma_start).
"""

import os
import sys

import numpy as np

for _p in ("/opt/trn_rl_repo",):
    if _p not in sys.path and os.path.isdir(_p):
        sys.path.insert(0, _p)

N_TOKENS = 4096
D_MODEL = 1024
D_HIDDEN = 4096
N_EXPERT = 8
TOP_K = 2
P = 128
KO = D_MODEL // P  # 8 contraction chunks for mm1
NCORES = 8
HB = D_HIDDEN // NCORES  # 512-wide hidden slice per core
HO = HB // P  # 4 h-chunks per core for mm2 contraction
DM = D_MODEL // P  # 8 output-partition chunks of Y^T
SLAB = 512  # max moving-dim per matmul (one PSUM bank of fp32)
WARMUP_MM = 10

# filled by kernel() for test harness introspection
last_results = None

_nc_cache = {}


def _even_split(L, cap=SLAB):
    n = -(-L // cap)
    base, extra = divmod(L, n)
    return [base + 1] * extra + [base] * (n - extra)


def _make_slabs(loads):
    """Slab plan: list of (expert, S, col0). Pairs are ordered expert-major;
    each expert segment is padded to a multiple of 4 columns (zero gate
    weight) so every slab start stays 4B-aligned for DMA. First slab is
    kept small so the first matmul's X/W DMA lead-in is short; last slab
    small to shrink the tail."""
    slabs = []
    col0 = 0
    segs = []  # (expert, col0, true_len)
    n_live = sum(1 for L in loads if L > 0)
    live_seen = 0
    for e, L in enumerate(loads):
        if L == 0:
            continue
        live_seen += 1
        Lp = -(-L // 4) * 4
        segs.append((e, col0, L))
        if live_seen == 1 and Lp > 256:
            parts = [128] + _even_split(Lp - 128)
        elif live_seen == n_live and Lp > 256:
            parts = _even_split(Lp - 128) + [128]
        else:
            parts = _even_split(Lp)
        for S in parts:
            slabs.append((e, S, col0))
            col0 += S
    return tuple(slabs), segs, col0


def _build_bass(slabs, cols):
    import concourse.mybir as mybir
    from concourse import bacc
    from concourse.tile import TileContext

    f16 = mybir.dt.float16
    f32 = mybir.dt.float32
    GELU = mybir.ActivationFunctionType.Gelu_apprx_tanh

    nc = bacc.Bacc("TRN2", target_bir_lowering=False, debug=False, num_devices=NCORES)

    SMAX = max(S for _, S, _ in slabs)

    x_d = nc.declare_dram_parameter("x", [P, KO * cols], f16, isOutput=False)
    w1_d = nc.declare_dram_parameter("w1", [N_EXPERT, P, KO, HB], f16, isOutput=False)
    w2_d = nc.declare_dram_parameter("w2", [N_EXPERT, P, HO, D_MODEL], f16, isOutput=False)
    b1_d = nc.declare_dram_parameter("b1", [P, N_EXPERT, HO], f32, isOutput=False)
    wb_d = nc.declare_dram_parameter("wb", [P, cols], f16, isOutput=False)
    out_d = nc.declare_dram_parameter("out", [DM, P, cols], f16, isOutput=True)

    # expert order of first use, for weight prefetch
    e_order = []
    first_slab_of = {}
    for si, (e, S, c0) in enumerate(slabs):
        if e not in first_slab_of:
            first_slab_of[e] = si
            e_order.append(e)

    with TileContext(nc) as tc:
        with (
            tc.tile_pool(name="wpool", bufs=1) as wpool,
            tc.tile_pool(name="xpool", bufs=2) as xpool,
            tc.tile_pool(name="hpool", bufs=2) as hpool,
            tc.tile_pool(name="ypool", bufs=3) as ypool,
            tc.tile_pool(name="ps1", bufs=3, space="PSUM") as ps1,
            tc.tile_pool(name="ps2", bufs=3, space="PSUM") as ps2,
        ):
            # PE warm-up: dependency-free matmuls keep PE busy (and HAM
            # warming) during the DMA lead-in for the first real matmul.
            warm = wpool.tile([P, 512], f16)
            nc.vector.memset(warm, 0.0)
            wps = ps1.tile([P, SMAX], mybir.dt.float32, tag="hps")
            for _ in range(WARMUP_MM):
                nc.tensor.matmul(
                    wps[:, :512], lhsT=warm[:, :P], rhs=warm, start=True, stop=True
                )

            b1_sb = wpool.tile([P, N_EXPERT, HO], f32)
            wb_sb = wpool.tile([P, cols], f16)
            w1_sb = wpool.tile([P, N_EXPERT, KO, HB], f16)
            w2_sb = wpool.tile([P, N_EXPERT, HO, D_MODEL], f16)

            # DMA issue order & engine assignment: descriptor generation is
            # ~0.7us per dma_start per sequencer, so spread issue across
            # otherwise-idle engines. gpsimd owns the 16MB weight stream,
            # vector owns the gate-weight row, scalar does b1, sync does
            # the per-slab X slabs (in-loop, self-pacing via pool sems).
            nc.scalar.dma_start(b1_sb, b1_d[:, :, :])
            # first expert's W1, split by h-chunk so the first chunk lands fast
            e0 = e_order[0]
            for ho in range(HO):
                nc.gpsimd.dma_start(
                    w1_sb[:, e0, :, ho * P : (ho + 1) * P],
                    w1_d[e0][:, :, ho * P : (ho + 1) * P],
                )
            # gate weights: first chunk covers the first few slabs
            wsplit = min(1024, cols)
            nc.scalar.dma_start(wb_sb[:, :wsplit], wb_d[:, :wsplit])
            if cols > wsplit:
                mid = wsplit + (cols - wsplit) // 2
                nc.scalar.dma_start(wb_sb[:, wsplit:mid], wb_d[:, wsplit:mid])
                nc.scalar.dma_start(wb_sb[:, mid:], wb_d[:, mid:])
            nc.gpsimd.dma_start(w2_sb[:, e0], w2_d[e0])
            if len(e_order) > 1:
                e1 = e_order[1]
                nc.gpsimd.dma_start(w1_sb[:, e1], w1_d[e1])
                nc.gpsimd.dma_start(w2_sb[:, e1], w2_d[e1])

            def mm1_slab(si):
                e, S, c0 = slabs[si]
                x_sb = xpool.tile([P, KO, SMAX], f16, tag="x", name="x_sb")[:, :, :S]
                x_src = x_d[:, KO * c0 : KO * (c0 + S)].rearrange(
                    "p (ko t) -> p ko t", t=S
                )
                nc.sync.dma_start(x_sb, x_src)
                # weight prefetch: at the first slab of expert-order i,
                # issue expert-order i+2 (i+1 was issued one boundary ago)
                if si == first_slab_of[e]:
                    oi = e_order.index(e)
                    if oi + 2 < len(e_order):
                        en = e_order[oi + 2]
                        nc.gpsimd.dma_start(w1_sb[:, en], w1_d[en])
                        nc.gpsimd.dma_start(w2_sb[:, en], w2_d[en])
                h_sb = hpool.tile([P, HO, SMAX], f16, tag="h", name="h_sb")[:, :, :S]
                for ho in range(HO):
                    hps = ps1.tile(
                        [P, SMAX], mybir.dt.float32, tag="hps", name="hps"
                    )[:, :S]
                    for k in range(KO):
                        nc.tensor.matmul(
                            hps,
                            lhsT=w1_sb[:, e, k, ho * P : (ho + 1) * P],
                            rhs=x_sb[:, k, :],
                            start=(k == 0),
                            stop=(k == KO - 1),
                        )
                    nc.scalar.activation(
                        h_sb[:, ho, :], hps, GELU, bias=b1_sb[:, e, ho : ho + 1]
                    )
                    # fold the per-pair gate weight into H (fp16, free dim)
                    nc.vector.tensor_mul(
                        h_sb[:, ho, :], h_sb[:, ho, :], wb_sb[:, c0 : c0 + S]
                    )
                return h_sb

            def mm2_slab(si, h_sb):
                e, S, c0 = slabs[si]
                for m in range(DM):
                    yps = ps2.tile(
                        [P, SMAX], mybir.dt.float32, tag="yps", name="yps"
                    )[:, :S]
                    for ho in range(HO):
                        nc.tensor.matmul(
                            yps,
                            lhsT=w2_sb[:, e, ho, m * P : (m + 1) * P],
                            rhs=h_sb[:, ho, :],
                            start=(ho == 0),
                            stop=(ho == HO - 1),
                        )
                    y_sb = ypool.tile([P, SMAX], f16, tag="y", name="y_sb")[:, :S]
                    # PSUM->SBUF fp16 copy, alternating engines for bandwidth
                    if m % 2 == 0:
                        nc.scalar.copy(y_sb, yps)
                    else:
                        nc.vector.tensor_copy(y_sb, yps)
                    nc.sync.dma_start(out_d[m, :, c0 : c0 + S], y_sb)

            # software pipeline: mm1(s+1) is emitted before mm2(s) so the
            # PE has a full slab of work while slab s's last gelu drains.
            h_prev = mm1_slab(0)
            for si in range(1, len(slabs)):
                h_cur = mm1_slab(si)
                mm2_slab(si - 1, h_prev)
                h_prev = h_cur
            mm2_slab(len(slabs) - 1, h_prev)
    nc.compile()
    return nc


def _route(moe_inp, Wg, bg):
    """Host gate: replicates NaiveGate (linear logits, top-2, softmax over the
    selected logits). Returns per-expert (token_idx, combine_weight)."""
    logits = moe_inp.astype(np.float32) @ Wg.astype(np.float32) + bg.astype(np.float32)
    order = np.argsort(-logits, axis=1, kind="stable")  # ties -> lower index first
    top_idx = order[:, :TOP_K]
    top_val = np.take_along_axis(logits, top_idx, axis=1)
    m = top_val.max(axis=1, keepdims=True)
    e = np.exp(top_val - m)
    gate = (e / e.sum(axis=1, keepdims=True)).astype(np.float32)
    toks, weights = [], []
    for ex in range(N_EXPERT):
        mask = top_idx == ex  # [N, K]; each token matches at most one slot
        t = np.nonzero(mask.any(axis=1))[0]
        w = gate[mask]  # row-major -> ascending token order, matches t
        toks.append(t)
        weights.append(w)
    return toks, weights


def kernel(**inputs):
    global last_results
    from concourse.bass_utils import run_bass_kernel_spmd

    moe_inp = np.asarray(inputs["moe_inp"], dtype=np.float32)
    Wg = np.asarray(inputs["Wg"], dtype=np.float32)
    bg = np.asarray(inputs["bg"], dtype=np.float32)
    W1 = np.asarray(inputs["W1"], dtype=np.float32)
    b1 = np.asarray(inputs["b1"], dtype=np.float32)
    W2 = np.asarray(inputs["W2"], dtype=np.float32)
    b2 = np.asarray(inputs["b2"], dtype=np.float32)

    toks, weights = _route(moe_inp, Wg, bg)
    loads = [len(t) for t in toks]
    slabs, segs, cols = _make_slabs(loads)

    if slabs not in _nc_cache:
        _nc_cache[slabs] = _build_bass(slabs, cols)
    nc = _nc_cache[slabs]

    # Dispatched X^T: pairs ordered expert-major, zero-padded per segment.
    xT = np.zeros((D_MODEL, cols), dtype=np.float16)
    wrow = np.zeros((cols,), dtype=np.float16)
    for e, c0, L in segs:
        xT[:, c0 : c0 + L] = moe_inp[toks[e]].T
        wrow[c0 : c0 + L] = weights[e]
    # per-slab partition-major blocks [P, KO*S], concatenated
    blocks = []
    for e, S, c0 in slabs:
        blocks.append(
            xT[:, c0 : c0 + S].reshape(KO, P, S).transpose(1, 0, 2).reshape(P, KO * S)
        )
    x_arr = np.ascontiguousarray(np.concatenate(blocks, axis=1))
    wb_arr = np.ascontiguousarray(np.broadcast_to(wrow, (P, cols)))

    in_maps = []
    for c in range(NCORES):
        lo, hi = c * HB, (c + 1) * HB
        w1_arr = np.ascontiguousarray(
            W1[:, :, lo:hi]
            .astype(np.float16)
            .reshape(N_EXPERT, KO, P, HB)
            .transpose(0, 2, 1, 3)
        )
        w2_arr = np.ascontiguousarray(
            W2[:, lo:hi, :]
            .astype(np.float16)
            .reshape(N_EXPERT, HO, P, D_MODEL)
            .transpose(0, 2, 1, 3)
        )
        b1_arr = np.ascontiguousarray(
            b1[:, lo:hi].reshape(N_EXPERT, HO, P).transpose(2, 0, 1)
        )
        in_maps.append(
            {"x": x_arr, "w1": w1_arr, "w2": w2_arr, "b1": b1_arr, "wb": wb_arr}
        )

    last_results = run_bass_kernel_spmd(nc, in_maps, core_ids=list(range(NCORES)))

    # host combine: sum the 8 hidden-slice partials, then scatter by segment
    yT = np.zeros((D_MODEL, cols), dtype=np.float32)
    for c in range(NCORES):
        yT += last_results.results[c]["out"].reshape(D_MODEL, cols).astype(np.float32)
    out = np.zeros((N_TOKENS, D_MODEL), dtype=np.float32)
    for e, c0, L in segs:
        out[toks[e]] += yT[:, c0 : c0 + L].T + weights[e][:, None] * b2[e][None, :]
    return out


if __name__ == "__main__":
    rng = np.random.default_rng(0)
    demo = {
        "moe_inp": rng.standard_normal((N_TOKENS, D_MODEL), dtype=np.float32),
        "attn_weights": rng.random((4, N_TOKENS, N_TOKENS), dtype=np.float32),
        "Wg": rng.standard_normal((D_MODEL, N_EXPERT), dtype=np.float32) / 32,
        "bg": np.zeros((N_EXPERT,), np.float32),
        "W1": rng.standard_normal((N_EXPERT, D_MODEL, D_HIDDEN), dtype=np.float32) / 32,
        "b1": np.zeros((N_EXPERT, D_HIDDEN), np.float32),
        "W2": rng.standard_normal((N_EXPERT, D_HIDDEN, D_MODEL), dtype=np.float32) / 64,
        "b2": np.zeros((N_EXPERT, D_MODEL), np.float32),
    }
    o = kernel(**demo)
    print(o.shape, o.dtype)


# revision 7
# speedup vs baseline: 1.2575x; 1.2575x over previous
"""FMoE forward (NaiveGate top-2, per-expert FFN, score-weighted combine) on 8 trn2 cores.

Strategy: tensor-parallel over D_HIDDEN. Each core holds a 512-wide hidden
slice of ALL 8 experts' W1/W2 (16MB resident in SBUF, same as one full
expert) and processes ALL 8192 dispatched token-expert pairs against its
slice -- perfect load balance by construction, no capacity padding to the
max expert load. The host computes the gate, orders pairs by expert
(segment structure identical on every core, so the kernel stays SPMD),
and sums the 8 partial Y outputs (each a 512-contraction slice of mm2).

Device kernel (per core, fp16 matmuls, fp32 accum):
  mm1: stationary = W1[e] slice chunk [128k, 128h], moving = X^T slab
       [128k, S] -> H^T chunk [128h, S] PSUM (8 k-chunks); ScalarE
       tanh-gelu (+b1), VectorE multiply by per-pair gate weight (fp16).
  mm2: stationary = W2[e] slice chunk [128h, 128d], moving = H^T chunk
       -> Y^T [128d, S] PSUM (4 h-chunks), fp16 copy out (scalar/vector
       alternating), DMA out.
Software pipeline: mm2 of slab s is emitted after mm1 of slab s+1 so the
PE never stalls on the last gelu of a slab. Weight/X/gate DMAs are issued
from separate engine sequencers (descriptor gen is ~0.7us per dma_start).
"""

import os
import sys

import numpy as np

for _p in ("/opt/trn_rl_repo",):
    if _p not in sys.path and os.path.isdir(_p):
        sys.path.insert(0, _p)

N_TOKENS = 4096
D_MODEL = 1024
D_HIDDEN = 4096
N_EXPERT = 8
TOP_K = 2
P = 128
KO = D_MODEL // P  # 8 contraction chunks for mm1
NCORES = 8
HB = D_HIDDEN // NCORES  # 512-wide hidden slice per core
HO = HB // P  # 4 h-chunks per core for mm2 contraction
DM = D_MODEL // P  # 8 output-partition chunks of Y^T
SLAB = 512  # max moving-dim per matmul (one PSUM bank of fp32)
WARMUP_MM = 10

# filled by kernel() for test harness introspection
last_results = None

_nc_cache = {}


def _even_split(L, cap=SLAB):
    """Split L (a multiple of 4) into even parts <= cap, each a multiple of 4."""
    q = L // 4
    n = -(-L // cap)
    base, extra = divmod(q, n)
    return [4 * (base + 1)] * extra + [4 * base] * (n - extra)


def _make_slabs(loads):
    """Slab plan: list of (expert, S, col0). Pairs are ordered expert-major;
    each expert segment is padded to a multiple of 4 columns (zero gate
    weight) so every slab start stays 4B-aligned for DMA. First slab is
    kept small so the first matmul's X/W DMA lead-in is short; last slab
    small to shrink the tail."""
    slabs = []
    col0 = 0
    segs = []  # (expert, col0, true_len)
    n_live = sum(1 for L in loads if L > 0)
    live_seen = 0
    for e, L in enumerate(loads):
        if L == 0:
            continue
        live_seen += 1
        Lp = -(-L // 4) * 4
        segs.append((e, col0, L))
        if live_seen == 1 and Lp > 256:
            parts = [128] + _even_split(Lp - 128)
        elif live_seen == n_live and Lp > 256:
            parts = _even_split(Lp - 128) + [128]
        else:
            parts = _even_split(Lp)
        for S in parts:
            slabs.append((e, S, col0))
            col0 += S
    return tuple(slabs), segs, col0


def _build_bass(slabs, cols):
    import concourse.mybir as mybir
    from concourse import bacc
    from concourse.tile import TileContext

    f16 = mybir.dt.float16
    f32 = mybir.dt.float32
    GELU = mybir.ActivationFunctionType.Gelu_apprx_tanh

    nc = bacc.Bacc("TRN2", target_bir_lowering=False, debug=False, num_devices=NCORES)

    SMAX = max(S for _, S, _ in slabs)

    x_d = nc.declare_dram_parameter("x", [P, KO * cols], f16, isOutput=False)
    w1_d = nc.declare_dram_parameter("w1", [N_EXPERT, P, KO, HB], f16, isOutput=False)
    w2_d = nc.declare_dram_parameter("w2", [N_EXPERT, P, HO, D_MODEL], f16, isOutput=False)
    b1_d = nc.declare_dram_parameter("b1", [P, N_EXPERT, HO], f32, isOutput=False)
    wb_d = nc.declare_dram_parameter("wb", [P, cols], f16, isOutput=False)
    out_d = nc.declare_dram_parameter("out", [DM, P, cols], f16, isOutput=True)

    # expert order of first use, for weight prefetch
    e_order = []
    first_slab_of = {}
    for si, (e, S, c0) in enumerate(slabs):
        if e not in first_slab_of:
            first_slab_of[e] = si
            e_order.append(e)

    with TileContext(nc) as tc:
        with (
            tc.tile_pool(name="wpool", bufs=1) as wpool,
            tc.tile_pool(name="xpool", bufs=2) as xpool,
            tc.tile_pool(name="hpool", bufs=2) as hpool,
            tc.tile_pool(name="ypool", bufs=2) as ypool,
            tc.tile_pool(name="ps1", bufs=3, space="PSUM") as ps1,
            tc.tile_pool(name="ps2", bufs=4, space="PSUM") as ps2,
        ):
            # PE warm-up: dependency-free matmuls keep PE busy (and HAM
            # warming) during the DMA lead-in for the first real matmul.
            warm = wpool.tile([P, 512], f16)
            nc.vector.memset(warm, 0.0)
            wps = ps1.tile([P, SMAX], mybir.dt.float32, tag="hps")
            for _ in range(WARMUP_MM):
                nc.tensor.matmul(
                    wps[:, :512], lhsT=warm[:, :P], rhs=warm, start=True, stop=True
                )

            b1_sb = wpool.tile([P, N_EXPERT, HO], f32)
            wb_sb = wpool.tile([P, cols], f16)
            w1_sb = wpool.tile([P, N_EXPERT, KO, HB], f16)
            w2_sb = wpool.tile([P, N_EXPERT, HO, D_MODEL], f16)

            # DMA issue order & engine assignment: descriptor generation is
            # ~0.7us per dma_start per sequencer, so spread issue across
            # otherwise-idle engines. gpsimd owns the 16MB weight stream,
            # vector owns the gate-weight row, scalar does b1, sync does
            # the per-slab X slabs (in-loop, self-pacing via pool sems).
            nc.scalar.dma_start(b1_sb, b1_d[:, :, :])
            # first expert's W1, split by h-chunk so the first chunk lands fast
            e0 = e_order[0]
            for ho in range(HO):
                nc.gpsimd.dma_start(
                    w1_sb[:, e0, :, ho * P : (ho + 1) * P],
                    w1_d[e0][:, :, ho * P : (ho + 1) * P],
                )
            # gate weights: first chunk covers the first few slabs
            wsplit = min(1024, cols)
            nc.scalar.dma_start(wb_sb[:, :wsplit], wb_d[:, :wsplit])
            if cols > wsplit:
                mid = wsplit + (cols - wsplit) // 2
                nc.scalar.dma_start(wb_sb[:, wsplit:mid], wb_d[:, wsplit:mid])
                nc.scalar.dma_start(wb_sb[:, mid:], wb_d[:, mid:])
            nc.gpsimd.dma_start(w2_sb[:, e0], w2_d[e0])
            if len(e_order) > 1:
                e1 = e_order[1]
                nc.gpsimd.dma_start(w1_sb[:, e1], w1_d[e1])
                nc.gpsimd.dma_start(w2_sb[:, e1], w2_d[e1])

            def mm1_slab(si):
                e, S, c0 = slabs[si]
                x_sb = xpool.tile([P, KO, SMAX], f16, tag="x", name="x_sb")[:, :, :S]
                x_src = x_d[:, KO * c0 : KO * (c0 + S)].rearrange(
                    "p (ko t) -> p ko t", t=S
                )
                # two dma_starts -> two parallel HW queues, halves the landing time
                nc.sync.dma_start(x_sb[:, : KO // 2, :], x_src[:, : KO // 2, :])
                nc.sync.dma_start(x_sb[:, KO // 2 :, :], x_src[:, KO // 2 :, :])
                # weight prefetch: at the first slab of expert-order i,
                # issue expert-order i+2 (i+1 was issued one boundary ago)
                if si == first_slab_of[e]:
                    oi = e_order.index(e)
                    if oi + 2 < len(e_order):
                        en = e_order[oi + 2]
                        nc.gpsimd.dma_start(w1_sb[:, en], w1_d[en])
                        nc.gpsimd.dma_start(w2_sb[:, en], w2_d[en])
                h_sb = hpool.tile([P, HO, SMAX], f16, tag="h", name="h_sb")[:, :, :S]
                for ho in range(HO):
                    hps = ps1.tile(
                        [P, SMAX], mybir.dt.float32, tag="hps", name="hps"
                    )[:, :S]
                    for k in range(KO):
                        nc.tensor.matmul(
                            hps,
                            lhsT=w1_sb[:, e, k, ho * P : (ho + 1) * P],
                            rhs=x_sb[:, k, :],
                            start=(k == 0),
                            stop=(k == KO - 1),
                        )
                    nc.scalar.activation(
                        h_sb[:, ho, :], hps, GELU, bias=b1_sb[:, e, ho : ho + 1]
                    )
                    # fold the per-pair gate weight into H (fp16, free dim)
                    nc.vector.tensor_mul(
                        h_sb[:, ho, :], h_sb[:, ho, :], wb_sb[:, c0 : c0 + S]
                    )
                return h_sb

            def mm2_slab(si, h_sb):
                e, S, c0 = slabs[si]
                y_all = ypool.tile([P, DM, SMAX], f16, tag="y", name="y_sb")[:, :, :S]
                for m in range(DM):
                    yps = ps2.tile(
                        [P, SMAX], mybir.dt.float32, tag="yps", name="yps"
                    )[:, :S]
                    for ho in range(HO):
                        nc.tensor.matmul(
                            yps,
                            lhsT=w2_sb[:, e, ho, m * P : (m + 1) * P],
                            rhs=h_sb[:, ho, :],
                            start=(ho == 0),
                            stop=(ho == HO - 1),
                        )
                    # PSUM->SBUF fp16 copy split across both engines: halves
                    # the copy latency so the 4-MM psum groups don't stall
                    h2 = (S // 8) * 4
                    nc.scalar.copy(y_all[:, m, :h2], yps[:, :h2])
                    nc.vector.tensor_copy(y_all[:, m, h2:], yps[:, h2:])
                # single out-DMA per slab, issued from gpsimd (sync is X-only)
                nc.gpsimd.dma_start(
                    out_d[:, :, c0 : c0 + S].rearrange("m p t -> p m t"), y_all
                )

            # software pipeline: mm1(s+1) is emitted before mm2(s) so the
            # PE has a full slab of work while slab s's last gelu drains.
            h_prev = mm1_slab(0)
            for si in range(1, len(slabs)):
                h_cur = mm1_slab(si)
                mm2_slab(si - 1, h_prev)
                h_prev = h_cur
            mm2_slab(len(slabs) - 1, h_prev)
    nc.compile()
    return nc


def _route(moe_inp, Wg, bg):
    """Host gate: replicates NaiveGate (linear logits, top-2, softmax over the
    selected logits). Returns per-expert (token_idx, combine_weight)."""
    logits = moe_inp.astype(np.float32) @ Wg.astype(np.float32) + bg.astype(np.float32)
    order = np.argsort(-logits, axis=1, kind="stable")  # ties -> lower index first
    top_idx = order[:, :TOP_K]
    top_val = np.take_along_axis(logits, top_idx, axis=1)
    m = top_val.max(axis=1, keepdims=True)
    e = np.exp(top_val - m)
    gate = (e / e.sum(axis=1, keepdims=True)).astype(np.float32)
    toks, weights = [], []
    for ex in range(N_EXPERT):
        mask = top_idx == ex  # [N, K]; each token matches at most one slot
        t = np.nonzero(mask.any(axis=1))[0]
        w = gate[mask]  # row-major -> ascending token order, matches t
        toks.append(t)
        weights.append(w)
    return toks, weights


def kernel(**inputs):
    global last_results
    from concourse.bass_utils import run_bass_kernel_spmd

    moe_inp = np.asarray(inputs["moe_inp"], dtype=np.float32)
    Wg = np.asarray(inputs["Wg"], dtype=np.float32)
    bg = np.asarray(inputs["bg"], dtype=np.float32)
    W1 = np.asarray(inputs["W1"], dtype=np.float32)
    b1 = np.asarray(inputs["b1"], dtype=np.float32)
    W2 = np.asarray(inputs["W2"], dtype=np.float32)
    b2 = np.asarray(inputs["b2"], dtype=np.float32)

    toks, weights = _route(moe_inp, Wg, bg)
    loads = [len(t) for t in toks]
    slabs, segs, cols = _make_slabs(loads)

    if slabs not in _nc_cache:
        _nc_cache[slabs] = _build_bass(slabs, cols)
    nc = _nc_cache[slabs]

    # Dispatched X^T: pairs ordered expert-major, zero-padded per segment.
    xT = np.zeros((D_MODEL, cols), dtype=np.float16)
    wrow = np.zeros((cols,), dtype=np.float16)
    for e, c0, L in segs:
        xT[:, c0 : c0 + L] = moe_inp[toks[e]].T
        wrow[c0 : c0 + L] = weights[e]
    # per-slab partition-major blocks [P, KO*S], concatenated
    blocks = []
    for e, S, c0 in slabs:
        blocks.append(
            xT[:, c0 : c0 + S].reshape(KO, P, S).transpose(1, 0, 2).reshape(P, KO * S)
        )
    x_arr = np.ascontiguousarray(np.concatenate(blocks, axis=1))
    wb_arr = np.ascontiguousarray(np.broadcast_to(wrow, (P, cols)))

    in_maps = []
    for c in range(NCORES):
        lo, hi = c * HB, (c + 1) * HB
        w1_arr = np.ascontiguousarray(
            W1[:, :, lo:hi]
            .astype(np.float16)
            .reshape(N_EXPERT, KO, P, HB)
            .transpose(0, 2, 1, 3)
        )
        w2_arr = np.ascontiguousarray(
            W2[:, lo:hi, :]
            .astype(np.float16)
            .reshape(N_EXPERT, HO, P, D_MODEL)
            .transpose(0, 2, 1, 3)
        )
        b1_arr = np.ascontiguousarray(
            b1[:, lo:hi].reshape(N_EXPERT, HO, P).transpose(2, 0, 1)
        )
        in_maps.append(
            {"x": x_arr, "w1": w1_arr, "w2": w2_arr, "b1": b1_arr, "wb": wb_arr}
        )

    last_results = run_bass_kernel_spmd(nc, in_maps, core_ids=list(range(NCORES)))

    # host combine: sum the 8 hidden-slice partials, then scatter by segment
    yT = np.zeros((D_MODEL, cols), dtype=np.float32)
    for c in range(NCORES):
        yT += last_results.results[c]["out"].reshape(D_MODEL, cols).astype(np.float32)
    out = np.zeros((N_TOKENS, D_MODEL), dtype=np.float32)
    for e, c0, L in segs:
        out[toks[e]] += yT[:, c0 : c0 + L].T + weights[e][:, None] * b2[e][None, :]
    return out


if __name__ == "__main__":
    rng = np.random.default_rng(0)
    demo = {
        "moe_inp": rng.standard_normal((N_TOKENS, D_MODEL), dtype=np.float32),
        "attn_weights": rng.random((4, N_TOKENS, N_TOKENS), dtype=np.float32),
        "Wg": rng.standard_normal((D_MODEL, N_EXPERT), dtype=np.float32) / 32,
        "bg": np.zeros((N_EXPERT,), np.float32),
        "W1": rng.standard_normal((N_EXPERT, D_MODEL, D_HIDDEN), dtype=np.float32) / 32,
        "b1": np.zeros((N_EXPERT, D_HIDDEN), np.float32),
        "W2": rng.standard_normal((N_EXPERT, D_HIDDEN, D_MODEL), dtype=np.float32) / 64,
        "b2": np.zeros((N_EXPERT, D_MODEL), np.float32),
    }
    o = kernel(**demo)
    print(o.shape, o.dtype)


# revision 10
# speedup vs baseline: 1.2871x; 1.0235x over previous
"""FMoE forward (NaiveGate top-2, per-expert FFN, score-weighted combine) on 8 trn2 cores.

Strategy: tensor-parallel over D_HIDDEN. Each core holds a 512-wide hidden
slice of ALL 8 experts' W1/W2 (16MB resident in SBUF, same as one full
expert) and processes ALL 8192 dispatched token-expert pairs against its
slice -- perfect load balance by construction, no capacity padding to the
max expert load. The host computes the gate, orders pairs by expert
(segment structure identical on every core, so the kernel stays SPMD),
and sums the 8 partial Y outputs (each a 512-contraction slice of mm2).

Device kernel (per core, fp16 matmuls, fp32 accum):
  mm1: stationary = W1[e] slice chunk [128k, 128h], moving = X^T slab
       [128k, S] -> H^T chunk [128h, S] PSUM (8 k-chunks); ScalarE
       tanh-gelu (+b1), VectorE multiply by per-pair gate weight (fp16).
  mm2: stationary = W2[e] slice chunk [128h, 128d], moving = H^T chunk
       -> Y^T [128d, S] PSUM (4 h-chunks), fp16 copy out (scalar/vector
       alternating), DMA out.
Software pipeline: mm2 of slab s is emitted after mm1 of slab s+1 so the
PE never stalls on the last gelu of a slab. Weight/X/gate DMAs are issued
from separate engine sequencers (descriptor gen is ~0.7us per dma_start).
"""

import os
import sys

import numpy as np

for _p in ("/opt/trn_rl_repo",):
    if _p not in sys.path and os.path.isdir(_p):
        sys.path.insert(0, _p)

N_TOKENS = 4096
D_MODEL = 1024
D_HIDDEN = 4096
N_EXPERT = 8
TOP_K = 2
P = 128
KO = D_MODEL // P  # 8 contraction chunks for mm1
NCORES = 8
HB = D_HIDDEN // NCORES  # 512-wide hidden slice per core
HO = HB // P  # 4 h-chunks per core for mm2 contraction
DM = D_MODEL // P  # 8 output-partition chunks of Y^T
SLAB = 512  # max moving-dim per matmul (one PSUM bank of fp32)
WARMUP_MM = 10

# filled by kernel() for test harness introspection
last_results = None

_nc_cache = {}


def _even_split(L, cap=SLAB):
    """Split L (a multiple of 4) into even parts <= cap, each a multiple of 4."""
    q = L // 4
    n = -(-L // cap)
    base, extra = divmod(q, n)
    return [4 * (base + 1)] * extra + [4 * base] * (n - extra)


def _make_slabs(loads):
    """Slab plan: list of (expert, S, col0). Pairs are ordered expert-major;
    each expert segment is padded to a multiple of 4 columns (zero gate
    weight) so every slab start stays 4B-aligned for DMA. First slab is
    kept small so the first matmul's X/W DMA lead-in is short; last slab
    small to shrink the tail."""
    slabs = []
    col0 = 0
    segs = []  # (expert, col0, true_len)
    n_live = sum(1 for L in loads if L > 0)
    live_seen = 0
    for e, L in enumerate(loads):
        if L == 0:
            continue
        live_seen += 1
        Lp = -(-L // 4) * 4
        segs.append((e, col0, L))
        if live_seen == 1 and Lp > 256:
            parts = [128] + _even_split(Lp - 128)
        elif live_seen == n_live and Lp > 256:
            parts = _even_split(Lp - 128) + [128]
        else:
            parts = _even_split(Lp)
        for S in parts:
            slabs.append((e, S, col0))
            col0 += S
    return tuple(slabs), segs, col0


def _build_bass(slabs, cols):
    import concourse.mybir as mybir
    from concourse import bacc
    from concourse.tile import TileContext

    f16 = mybir.dt.float16
    f32 = mybir.dt.float32
    GELU = mybir.ActivationFunctionType.Gelu_apprx_tanh

    nc = bacc.Bacc("TRN2", target_bir_lowering=False, debug=False, num_devices=NCORES)

    SMAX = max(S for _, S, _ in slabs)

    x_d = nc.declare_dram_parameter("x", [P, KO * cols], f16, isOutput=False)
    w1_d = nc.declare_dram_parameter("w1", [N_EXPERT, P, KO, HB], f16, isOutput=False)
    w2_d = nc.declare_dram_parameter("w2", [N_EXPERT, P, HO, D_MODEL], f16, isOutput=False)
    b1_d = nc.declare_dram_parameter("b1", [P, N_EXPERT, HO], f32, isOutput=False)
    wb_d = nc.declare_dram_parameter("wb", [P, cols], f16, isOutput=False)
    # slab-blocked output: per slab a contiguous [P, DM*S] block at DM*col0,
    # so each out-DMA is one 8KB-per-partition contiguous run (cheap descriptor)
    out_d = nc.declare_dram_parameter("out", [P, DM * cols], f16, isOutput=True)

    # expert order of first use, for weight prefetch
    e_order = []
    first_slab_of = {}
    for si, (e, S, c0) in enumerate(slabs):
        if e not in first_slab_of:
            first_slab_of[e] = si
            e_order.append(e)

    with TileContext(nc) as tc:
        with (
            tc.tile_pool(name="wpool", bufs=1) as wpool,
            tc.tile_pool(name="xpool", bufs=2) as xpool,
            tc.tile_pool(name="hpool", bufs=2) as hpool,
            tc.tile_pool(name="ypool", bufs=2) as ypool,
            tc.tile_pool(name="ps1", bufs=3, space="PSUM") as ps1,
            tc.tile_pool(name="ps2", bufs=4, space="PSUM") as ps2,
        ):
            # PE warm-up: dependency-free matmuls keep PE busy (and HAM
            # warming) during the DMA lead-in for the first real matmul.
            warm = wpool.tile([P, 512], f16)
            nc.vector.memset(warm, 0.0)
            wps = ps1.tile([P, SMAX], mybir.dt.float32, tag="hps")
            for _ in range(WARMUP_MM):
                nc.tensor.matmul(
                    wps[:, :512], lhsT=warm[:, :P], rhs=warm, start=True, stop=True
                )

            b1_sb = wpool.tile([P, N_EXPERT, HO], f32)
            wb_sb = wpool.tile([P, cols], f16)
            w1_sb = wpool.tile([P, N_EXPERT, KO, HB], f16)
            w2_sb = wpool.tile([P, N_EXPERT, HO, D_MODEL], f16)

            # DMA issue order & engine assignment: descriptor generation is
            # ~0.7us per dma_start per sequencer, so spread issue across
            # otherwise-idle engines. gpsimd owns the 16MB weight stream,
            # vector owns the gate-weight row, scalar does b1, sync does
            # the per-slab X slabs (in-loop, self-pacing via pool sems).
            nc.scalar.dma_start(b1_sb, b1_d[:, :, :])
            # first expert's W1, split by h-chunk so the first chunk lands fast
            e0 = e_order[0]
            for ho in range(HO):
                nc.gpsimd.dma_start(
                    w1_sb[:, e0, :, ho * P : (ho + 1) * P],
                    w1_d[e0][:, :, ho * P : (ho + 1) * P],
                )
            # gate weights: first chunk covers the first few slabs
            wsplit = min(1024, cols)
            nc.scalar.dma_start(wb_sb[:, :wsplit], wb_d[:, :wsplit])
            if cols > wsplit:
                mid = wsplit + (cols - wsplit) // 2
                nc.scalar.dma_start(wb_sb[:, wsplit:mid], wb_d[:, wsplit:mid])
                nc.scalar.dma_start(wb_sb[:, mid:], wb_d[:, mid:])
            nc.gpsimd.dma_start(w2_sb[:, e0], w2_d[e0])
            if len(e_order) > 1:
                e1 = e_order[1]
                nc.gpsimd.dma_start(w1_sb[:, e1], w1_d[e1])
                nc.gpsimd.dma_start(w2_sb[:, e1], w2_d[e1])

            def mm1_slab(si):
                e, S, c0 = slabs[si]
                x_sb = xpool.tile([P, KO, SMAX], f16, tag="x", name="x_sb")[:, :, :S]
                x_src = x_d[:, KO * c0 : KO * (c0 + S)].rearrange(
                    "p (ko t) -> p ko t", t=S
                )
                # two dma_starts -> two parallel HW queues, halves the landing time
                nc.sync.dma_start(x_sb[:, : KO // 2, :], x_src[:, : KO // 2, :])
                nc.sync.dma_start(x_sb[:, KO // 2 :, :], x_src[:, KO // 2 :, :])
                # weight prefetch: at the first slab of expert-order i,
                # issue expert-order i+2 (i+1 was issued one boundary ago)
                if si == first_slab_of[e]:
                    oi = e_order.index(e)
                    if oi + 2 < len(e_order):
                        en = e_order[oi + 2]
                        nc.gpsimd.dma_start(w1_sb[:, en], w1_d[en])
                        nc.gpsimd.dma_start(w2_sb[:, en], w2_d[en])
                h_sb = hpool.tile([P, HO, SMAX], f16, tag="h", name="h_sb")[:, :, :S]
                for ho in range(HO):
                    hps = ps1.tile(
                        [P, SMAX], mybir.dt.float32, tag="hps", name="hps"
                    )[:, :S]
                    for k in range(KO):
                        nc.tensor.matmul(
                            hps,
                            lhsT=w1_sb[:, e, k, ho * P : (ho + 1) * P],
                            rhs=x_sb[:, k, :],
                            start=(k == 0),
                            stop=(k == KO - 1),
                        )
                    nc.scalar.activation(
                        h_sb[:, ho, :], hps, GELU, bias=b1_sb[:, e, ho : ho + 1]
                    )
                    # fold the per-pair gate weight into H (fp16, free dim)
                    nc.vector.tensor_mul(
                        h_sb[:, ho, :], h_sb[:, ho, :], wb_sb[:, c0 : c0 + S]
                    )
                return h_sb

            def mm2_slab(si, h_sb):
                e, S, c0 = slabs[si]
                y_all = ypool.tile([P, DM, SMAX], f16, tag="y", name="y_sb")[:, :, :S]
                for m in range(DM):
                    yps = ps2.tile(
                        [P, SMAX], mybir.dt.float32, tag="yps", name="yps"
                    )[:, :S]
                    for ho in range(HO):
                        nc.tensor.matmul(
                            yps,
                            lhsT=w2_sb[:, e, ho, m * P : (m + 1) * P],
                            rhs=h_sb[:, ho, :],
                            start=(ho == 0),
                            stop=(ho == HO - 1),
                        )
                    # PSUM->SBUF fp16 copy split across both engines: halves
                    # the copy latency so the 4-MM psum groups don't stall
                    h2 = (S // 8) * 4
                    nc.scalar.copy(y_all[:, m, :h2], yps[:, :h2])
                    nc.vector.tensor_copy(y_all[:, m, h2:], yps[:, h2:])
                # single out-DMA per slab, issued from gpsimd (sync is X-only)
                out_dst = out_d[:, DM * c0 : DM * (c0 + S)].rearrange(
                    "p (m t) -> p m t", t=S
                )
                nc.gpsimd.dma_start(out_dst, y_all)

            # software pipeline: mm1(s+1) is emitted before mm2(s) so the
            # PE has a full slab of work while slab s's last gelu drains.
            h_prev = mm1_slab(0)
            for si in range(1, len(slabs)):
                h_cur = mm1_slab(si)
                mm2_slab(si - 1, h_prev)
                h_prev = h_cur
            mm2_slab(len(slabs) - 1, h_prev)
    nc.compile()
    return nc


def _route(moe_inp, Wg, bg):
    """Host gate: replicates NaiveGate (linear logits, top-2, softmax over the
    selected logits). Returns per-expert (token_idx, combine_weight)."""
    logits = moe_inp.astype(np.float32) @ Wg.astype(np.float32) + bg.astype(np.float32)
    order = np.argsort(-logits, axis=1, kind="stable")  # ties -> lower index first
    top_idx = order[:, :TOP_K]
    top_val = np.take_along_axis(logits, top_idx, axis=1)
    m = top_val.max(axis=1, keepdims=True)
    e = np.exp(top_val - m)
    gate = (e / e.sum(axis=1, keepdims=True)).astype(np.float32)
    toks, weights = [], []
    for ex in range(N_EXPERT):
        mask = top_idx == ex  # [N, K]; each token matches at most one slot
        t = np.nonzero(mask.any(axis=1))[0]
        w = gate[mask]  # row-major -> ascending token order, matches t
        toks.append(t)
        weights.append(w)
    return toks, weights


def kernel(**inputs):
    global last_results
    from concourse.bass_utils import run_bass_kernel_spmd

    moe_inp = np.asarray(inputs["moe_inp"], dtype=np.float32)
    Wg = np.asarray(inputs["Wg"], dtype=np.float32)
    bg = np.asarray(inputs["bg"], dtype=np.float32)
    W1 = np.asarray(inputs["W1"], dtype=np.float32)
    b1 = np.asarray(inputs["b1"], dtype=np.float32)
    W2 = np.asarray(inputs["W2"], dtype=np.float32)
    b2 = np.asarray(inputs["b2"], dtype=np.float32)

    toks, weights = _route(moe_inp, Wg, bg)
    loads = [len(t) for t in toks]
    slabs, segs, cols = _make_slabs(loads)

    if slabs not in _nc_cache:
        _nc_cache[slabs] = _build_bass(slabs, cols)
    nc = _nc_cache[slabs]

    # Dispatched X^T: pairs ordered expert-major, zero-padded per segment.
    xT = np.zeros((D_MODEL, cols), dtype=np.float16)
    wrow = np.zeros((cols,), dtype=np.float16)
    for e, c0, L in segs:
        xT[:, c0 : c0 + L] = moe_inp[toks[e]].T
        wrow[c0 : c0 + L] = weights[e]
    # per-slab partition-major blocks [P, KO*S], concatenated
    blocks = []
    for e, S, c0 in slabs:
        blocks.append(
            xT[:, c0 : c0 + S].reshape(KO, P, S).transpose(1, 0, 2).reshape(P, KO * S)
        )
    x_arr = np.ascontiguousarray(np.concatenate(blocks, axis=1))
    wb_arr = np.ascontiguousarray(np.broadcast_to(wrow, (P, cols)))

    in_maps = []
    for c in range(NCORES):
        lo, hi = c * HB, (c + 1) * HB
        w1_arr = np.ascontiguousarray(
            W1[:, :, lo:hi]
            .astype(np.float16)
            .reshape(N_EXPERT, KO, P, HB)
            .transpose(0, 2, 1, 3)
        )
        w2_arr = np.ascontiguousarray(
            W2[:, lo:hi, :]
            .astype(np.float16)
            .reshape(N_EXPERT, HO, P, D_MODEL)
            .transpose(0, 2, 1, 3)
        )
        b1_arr = np.ascontiguousarray(
            b1[:, lo:hi].reshape(N_EXPERT, HO, P).transpose(2, 0, 1)
        )
        in_maps.append(
            {"x": x_arr, "w1": w1_arr, "w2": w2_arr, "b1": b1_arr, "wb": wb_arr}
        )

    last_results = run_bass_kernel_spmd(nc, in_maps, core_ids=list(range(NCORES)))

    # host combine: sum the 8 hidden-slice partials, decode the slab-blocked
    # layout once, then scatter by segment
    raw = np.zeros((P, DM * cols), dtype=np.float32)
    for c in range(NCORES):
        raw += last_results.results[c]["out"].astype(np.float32)
    yT = np.empty((D_MODEL, cols), dtype=np.float32)
    for e, S, c0 in slabs:
        blk = raw[:, DM * c0 : DM * (c0 + S)].reshape(P, DM, S)
        yT[:, c0 : c0 + S] = blk.transpose(1, 0, 2).reshape(D_MODEL, S)
    out = np.zeros((N_TOKENS, D_MODEL), dtype=np.float32)
    for e, c0, L in segs:
        out[toks[e]] += yT[:, c0 : c0 + L].T + weights[e][:, None] * b2[e][None, :]
    return out


if __name__ == "__main__":
    rng = np.random.default_rng(0)
    demo = {
        "moe_inp": rng.standard_normal((N_TOKENS, D_MODEL), dtype=np.float32),
        "attn_weights": rng.random((4, N_TOKENS, N_TOKENS), dtype=np.float32),
        "Wg": rng.standard_normal((D_MODEL, N_EXPERT), dtype=np.float32) / 32,
        "bg": np.zeros((N_EXPERT,), np.float32),
        "W1": rng.standard_normal((N_EXPERT, D_MODEL, D_HIDDEN), dtype=np.float32) / 32,
        "b1": np.zeros((N_EXPERT, D_HIDDEN), np.float32),
        "W2": rng.standard_normal((N_EXPERT, D_HIDDEN, D_MODEL), dtype=np.float32) / 64,
        "b2": np.zeros((N_EXPERT, D_MODEL), np.float32),
    }
    o = kernel(**demo)
    print(o.shape, o.dtype)


# revision 16
# speedup vs baseline: 1.3962x; 1.0848x over previous
"""FMoE forward (NaiveGate top-2, per-expert FFN, score-weighted combine) on 8 trn2 cores.

Strategy: tensor-parallel over D_HIDDEN. Each core holds a 512-wide hidden
slice of ALL 8 experts' W1/W2 (16MB resident in SBUF, same as one full
expert) and processes ALL 8192 dispatched token-expert pairs against its
slice -- perfect load balance by construction, no capacity padding to the
max expert load. The host computes the gate, orders pairs by expert
(segment structure identical on every core, so the kernel stays SPMD),
and sums the 8 partial Y outputs (each a 512-contraction slice of mm2).

Device kernel (per core, fp16 matmuls, fp32 accum):
  mm1: stationary = W1[e] slice chunk [128k, 128h], moving = X^T slab
       [128k, S] -> H^T chunk [128h, S] PSUM (8 k-chunks); ScalarE
       tanh-gelu (+b1), VectorE multiply by per-pair gate weight (fp16).
  mm2: stationary = W2[e] slice chunk [128h, 128d], moving = H^T chunk
       -> Y^T [128d, S] PSUM (4 h-chunks), fp16 copy out (scalar/vector
       alternating), DMA out.
Software pipeline: mm2 of slab s is emitted after mm1 of slab s+1 so the
PE never stalls on the last gelu of a slab. Weight/X/gate DMAs are issued
from separate engine sequencers (descriptor gen is ~0.7us per dma_start).
"""

import os
import sys

import numpy as np

for _p in ("/opt/trn_rl_repo",):
    if _p not in sys.path and os.path.isdir(_p):
        sys.path.insert(0, _p)

N_TOKENS = 4096
D_MODEL = 1024
D_HIDDEN = 4096
N_EXPERT = 8
TOP_K = 2
P = 128
KO = D_MODEL // P  # 8 contraction chunks for mm1
NCORES = 8
HB = D_HIDDEN // NCORES  # 512-wide hidden slice per core
HO = HB // P  # 4 h-chunks per core for mm2 contraction
DM = D_MODEL // P  # 8 output-partition chunks of Y^T
SLAB = 512  # max moving-dim per matmul (one PSUM bank of fp32)
WARMUP_MM = 10

# filled by kernel() for test harness introspection
last_results = None

_nc_cache = {}


def _even_split(L, cap=SLAB):
    """Split L (a multiple of 4) into even parts <= cap, each a multiple of 4."""
    q = L // 4
    n = -(-L // cap)
    base, extra = divmod(q, n)
    return [4 * (base + 1)] * extra + [4 * base] * (n - extra)


def _make_slabs(loads):
    """Slab plan: list of (expert, S, col0). Pairs are ordered expert-major;
    each expert segment is padded to a multiple of 4 columns (zero gate
    weight) so every slab start stays 4B-aligned for DMA. First slab is
    kept small so the first matmul's X/W DMA lead-in is short; last slab
    small to shrink the tail."""
    slabs = []
    col0 = 0
    segs = []  # (expert, col0, true_len)
    n_live = sum(1 for L in loads if L > 0)
    live_seen = 0
    for e, L in enumerate(loads):
        if L == 0:
            continue
        live_seen += 1
        Lp = -(-L // 4) * 4
        segs.append((e, col0, L))
        if live_seen == 1 and Lp > 256:
            parts = [128] + _even_split(Lp - 128)
        elif live_seen == n_live and Lp > 768:
            # taper the tail so the final copies+out-DMA flush is short
            parts = _even_split(Lp - 384) + [256, 128]
        elif live_seen == n_live and Lp > 256:
            parts = _even_split(Lp - 128) + [128]
        else:
            parts = _even_split(Lp)
        for S in parts:
            slabs.append((e, S, col0))
            col0 += S
    return tuple(slabs), segs, col0


def _build_bass(slabs, cols):
    import concourse.mybir as mybir
    from concourse import bacc
    from concourse.tile import TileContext

    f16 = mybir.dt.float16
    f32 = mybir.dt.float32
    GELU = mybir.ActivationFunctionType.Gelu_apprx_tanh

    nc = bacc.Bacc("TRN2", target_bir_lowering=False, debug=False, num_devices=NCORES)

    SMAX = max(S for _, S, _ in slabs)

    x_d = nc.declare_dram_parameter("x", [P, KO * cols], f16, isOutput=False)
    w1_d = nc.declare_dram_parameter("w1", [N_EXPERT, P, KO, HB], f16, isOutput=False)
    w2_d = nc.declare_dram_parameter("w2", [N_EXPERT, P, HO, D_MODEL], f16, isOutput=False)
    b1_d = nc.declare_dram_parameter("b1", [P, N_EXPERT, HO], f32, isOutput=False)
    wb_d = nc.declare_dram_parameter("wb", [P, cols], f16, isOutput=False)
    # slab-blocked output: per slab a contiguous [P, DM*S] block at DM*col0,
    # so each out-DMA is one 8KB-per-partition contiguous run (cheap descriptor)
    out_d = nc.declare_dram_parameter("out", [P, DM * cols], f16, isOutput=True)

    # expert order of first use, for weight prefetch
    e_order = []
    first_slab_of = {}
    for si, (e, S, c0) in enumerate(slabs):
        if e not in first_slab_of:
            first_slab_of[e] = si
            e_order.append(e)

    with TileContext(nc) as tc:
        with (
            tc.tile_pool(name="wpool", bufs=1) as wpool,
            tc.tile_pool(name="xpool", bufs=2) as xpool,
            tc.tile_pool(name="wbpool", bufs=3) as wbpool,
            tc.tile_pool(name="hpool", bufs=2) as hpool,
            tc.tile_pool(name="ypool", bufs=2) as ypool,
            tc.tile_pool(name="ps1", bufs=3, space="PSUM") as ps1,
            tc.tile_pool(name="ps2", bufs=4, space="PSUM") as ps2,
        ):
            # PE warm-up: dependency-free matmuls keep PE busy (and HAM
            # warming) during the DMA lead-in for the first real matmul.
            warm = wpool.tile([P, 512], f16)
            nc.vector.memset(warm, 0.0)
            wps = ps1.tile([P, SMAX], mybir.dt.float32, tag="hps")
            for _ in range(WARMUP_MM):
                nc.tensor.matmul(
                    wps[:, :512], lhsT=warm[:, :P], rhs=warm, start=True, stop=True
                )

            b1_sb = wpool.tile([P, N_EXPERT, HO], f32)
            w1_sb = wpool.tile([P, N_EXPERT, KO, HB], f16)
            w2_sb = wpool.tile([P, N_EXPERT, HO, D_MODEL], f16)

            # Weight stream in strict first-need order; ~1MB piece per slab
            # boundary so the early HBM window (8 cores all loading) isn't
            # oversubscribed. gpsimd owns the stream; sync owns X; scalar
            # owns b1 + per-slab gate rows.
            wq = []
            for oi, e in enumerate(e_order):
                wq.append(("w1", e))
                wq.append(("w2", e))

            def issue_weight():
                if not wq:
                    return
                kind, e = wq.pop(0)
                if kind == "w1":
                    nc.gpsimd.dma_start(w1_sb[:, e], w1_d[e])
                else:
                    nc.gpsimd.dma_start(w2_sb[:, e], w2_d[e])

            nc.scalar.dma_start(b1_sb, b1_d[:, :, :])
            # first expert's W1 split by h-chunk so the first chunk lands fast
            e0 = e_order[0]
            wq.pop(0)
            for ho in range(HO):
                nc.gpsimd.dma_start(
                    w1_sb[:, e0, :, ho * P : (ho + 1) * P],
                    w1_d[e0][:, :, ho * P : (ho + 1) * P],
                )
            # W2[e0] in halves (mm2 consumes m-chunks in order), then W1[e1]
            wq.pop(0)
            nc.gpsimd.dma_start(w2_sb[:, e0, :, : D_MODEL // 2], w2_d[e0][:, :, : D_MODEL // 2])
            nc.gpsimd.dma_start(w2_sb[:, e0, :, D_MODEL // 2 :], w2_d[e0][:, :, D_MODEL // 2 :])
            issue_weight()

            def mm1_slab(si):
                e, S, c0 = slabs[si]
                x_sb = xpool.tile([P, KO, SMAX], f16, tag="x", name="x_sb")[:, :, :S]
                x_src = x_d[:, KO * c0 : KO * (c0 + S)].rearrange(
                    "p (ko t) -> p ko t", t=S
                )
                # two dma_starts -> two parallel HW queues, halves the landing time
                nc.sync.dma_start(x_sb[:, : KO // 2, :], x_src[:, : KO // 2, :])
                nc.sync.dma_start(x_sb[:, KO // 2 :, :], x_src[:, KO // 2 :, :])
                if si >= 1:
                    issue_weight()
                wb_t = wbpool.tile([P, SMAX], f16, tag="wb", name="wb_t")[:, :S]
                nc.scalar.dma_start(wb_t, wb_d[:, c0 : c0 + S])
                h_sb = hpool.tile([P, HO, SMAX], f16, tag="h", name="h_sb")[:, :, :S]
                for ho in range(HO):
                    hps = ps1.tile(
                        [P, SMAX], mybir.dt.float32, tag="hps", name="hps"
                    )[:, :S]
                    for k in range(KO):
                        nc.tensor.matmul(
                            hps,
                            lhsT=w1_sb[:, e, k, ho * P : (ho + 1) * P],
                            rhs=x_sb[:, k, :],
                            start=(k == 0),
                            stop=(k == KO - 1),
                        )
                    nc.scalar.activation(
                        h_sb[:, ho, :], hps, GELU, bias=b1_sb[:, e, ho : ho + 1]
                    )
                    # fold the per-pair gate weight into H (fp16, free dim)
                    nc.vector.tensor_mul(h_sb[:, ho, :], h_sb[:, ho, :], wb_t)
                return h_sb

            def mm2_slab(si, h_sb):
                e, S, c0 = slabs[si]
                last2 = si >= len(slabs) - 2
                y_all = ypool.tile([P, DM, SMAX], f16, tag="y", name="y_sb")[:, :, :S]
                out_dst = out_d[:, DM * c0 : DM * (c0 + S)].rearrange(
                    "p (m t) -> p m t", t=S
                )
                for m in range(DM):
                    yps = ps2.tile(
                        [P, SMAX], mybir.dt.float32, tag="yps", name="yps"
                    )[:, :S]
                    for ho in range(HO):
                        nc.tensor.matmul(
                            yps,
                            lhsT=w2_sb[:, e, ho, m * P : (m + 1) * P],
                            rhs=h_sb[:, ho, :],
                            start=(ho == 0),
                            stop=(ho == HO - 1),
                        )
                    # PSUM->SBUF fp16 copy split across both engines: halves
                    # the copy latency so the 4-MM psum groups don't stall
                    h2 = (S // 8) * 4
                    nc.scalar.copy(y_all[:, m, :h2], yps[:, :h2])
                    nc.vector.tensor_copy(y_all[:, m, h2:], yps[:, h2:])
                    # tail slabs: flush halves from the (idle-by-then) sync
                    # engine so the final out-DMA overlaps the last matmuls
                    if last2 and m == DM // 2 - 1:
                        nc.sync.dma_start(out_dst[:, : DM // 2], y_all[:, : DM // 2])
                if last2:
                    nc.sync.dma_start(out_dst[:, DM // 2 :], y_all[:, DM // 2 :])
                else:
                    # single out-DMA per slab, issued from gpsimd (sync is X-only)
                    nc.gpsimd.dma_start(out_dst, y_all)

            # software pipeline: mm1(s+1) is emitted before mm2(s) so the
            # PE has a full slab of work while slab s's last gelu drains.
            h_prev = mm1_slab(0)
            for si in range(1, len(slabs)):
                h_cur = mm1_slab(si)
                mm2_slab(si - 1, h_prev)
                h_prev = h_cur
            mm2_slab(len(slabs) - 1, h_prev)
    nc.compile()
    return nc


def _route(moe_inp, Wg, bg):
    """Host gate: replicates NaiveGate (linear logits, top-2, softmax over the
    selected logits). Returns per-expert (token_idx, combine_weight)."""
    logits = moe_inp.astype(np.float32) @ Wg.astype(np.float32) + bg.astype(np.float32)
    order = np.argsort(-logits, axis=1, kind="stable")  # ties -> lower index first
    top_idx = order[:, :TOP_K]
    top_val = np.take_along_axis(logits, top_idx, axis=1)
    m = top_val.max(axis=1, keepdims=True)
    e = np.exp(top_val - m)
    gate = (e / e.sum(axis=1, keepdims=True)).astype(np.float32)
    toks, weights = [], []
    for ex in range(N_EXPERT):
        mask = top_idx == ex  # [N, K]; each token matches at most one slot
        t = np.nonzero(mask.any(axis=1))[0]
        w = gate[mask]  # row-major -> ascending token order, matches t
        toks.append(t)
        weights.append(w)
    return toks, weights


def kernel(**inputs):
    global last_results
    from concourse.bass_utils import run_bass_kernel_spmd

    moe_inp = np.asarray(inputs["moe_inp"], dtype=np.float32)
    Wg = np.asarray(inputs["Wg"], dtype=np.float32)
    bg = np.asarray(inputs["bg"], dtype=np.float32)
    W1 = np.asarray(inputs["W1"], dtype=np.float32)
    b1 = np.asarray(inputs["b1"], dtype=np.float32)
    W2 = np.asarray(inputs["W2"], dtype=np.float32)
    b2 = np.asarray(inputs["b2"], dtype=np.float32)

    toks, weights = _route(moe_inp, Wg, bg)
    loads = [len(t) for t in toks]
    slabs, segs, cols = _make_slabs(loads)

    if slabs not in _nc_cache:
        _nc_cache[slabs] = _build_bass(slabs, cols)
    nc = _nc_cache[slabs]

    # Dispatched X^T: pairs ordered expert-major, zero-padded per segment.
    xT = np.zeros((D_MODEL, cols), dtype=np.float16)
    wrow = np.zeros((cols,), dtype=np.float16)
    for e, c0, L in segs:
        xT[:, c0 : c0 + L] = moe_inp[toks[e]].T
        wrow[c0 : c0 + L] = weights[e]
    # per-slab partition-major blocks [P, KO*S], concatenated
    blocks = []
    for e, S, c0 in slabs:
        blocks.append(
            xT[:, c0 : c0 + S].reshape(KO, P, S).transpose(1, 0, 2).reshape(P, KO * S)
        )
    x_arr = np.ascontiguousarray(np.concatenate(blocks, axis=1))
    wb_arr = np.ascontiguousarray(np.broadcast_to(wrow, (P, cols)))

    in_maps = []
    for c in range(NCORES):
        lo, hi = c * HB, (c + 1) * HB
        w1_arr = np.ascontiguousarray(
            W1[:, :, lo:hi]
            .astype(np.float16)
            .reshape(N_EXPERT, KO, P, HB)
            .transpose(0, 2, 1, 3)
        )
        w2_arr = np.ascontiguousarray(
            W2[:, lo:hi, :]
            .astype(np.float16)
            .reshape(N_EXPERT, HO, P, D_MODEL)
            .transpose(0, 2, 1, 3)
        )
        b1_arr = np.ascontiguousarray(
            b1[:, lo:hi].reshape(N_EXPERT, HO, P).transpose(2, 0, 1)
        )
        in_maps.append(
            {"x": x_arr, "w1": w1_arr, "w2": w2_arr, "b1": b1_arr, "wb": wb_arr}
        )

    last_results = run_bass_kernel_spmd(nc, in_maps, core_ids=list(range(NCORES)))

    # host combine: sum the 8 hidden-slice partials, decode the slab-blocked
    # layout once, then scatter by segment
    raw = np.zeros((P, DM * cols), dtype=np.float32)
    for c in range(NCORES):
        raw += last_results.results[c]["out"].astype(np.float32)
    yT = np.empty((D_MODEL, cols), dtype=np.float32)
    for e, S, c0 in slabs:
        blk = raw[:, DM * c0 : DM * (c0 + S)].reshape(P, DM, S)
        yT[:, c0 : c0 + S] = blk.transpose(1, 0, 2).reshape(D_MODEL, S)
    out = np.zeros((N_TOKENS, D_MODEL), dtype=np.float32)
    for e, c0, L in segs:
        out[toks[e]] += yT[:, c0 : c0 + L].T + weights[e][:, None] * b2[e][None, :]
    return out


if __name__ == "__main__":
    rng = np.random.default_rng(0)
    demo = {
        "moe_inp": rng.standard_normal((N_TOKENS, D_MODEL), dtype=np.float32),
        "attn_weights": rng.random((4, N_TOKENS, N_TOKENS), dtype=np.float32),
        "Wg": rng.standard_normal((D_MODEL, N_EXPERT), dtype=np.float32) / 32,
        "bg": np.zeros((N_EXPERT,), np.float32),
        "W1": rng.standard_normal((N_EXPERT, D_MODEL, D_HIDDEN), dtype=np.float32) / 32,
        "b1": np.zeros((N_EXPERT, D_HIDDEN), np.float32),
        "W2": rng.standard_normal((N_EXPERT, D_HIDDEN, D_MODEL), dtype=np.float32) / 64,
        "b2": np.zeros((N_EXPERT, D_MODEL), np.float32),
    }
    o = kernel(**demo)
    print(o.shape, o.dtype)


# revision 20
# speedup vs baseline: 1.4289x; 1.0234x over previous
"""FMoE forward (NaiveGate top-2, per-expert FFN, score-weighted combine) on 8 trn2 cores.

Strategy: hybrid expert-parallel x tensor-parallel. Cores split into 2
groups of 4; each group owns 4 experts; within a group each core holds a
1024-wide hidden slice of its 4 experts' W1/W2 (16MB resident in SBUF) and
processes ALL of the group's dispatched token-expert pairs against its
slice. Per-core DMA is ~33MB (vs ~49MB for pure 8-way TP), which keeps
the HBM stream comfortably under the per-core budget, and load balance is
near-perfect: the SPMD slab plan uses the elementwise max of the two
groups' (sorted) segment lengths, with the expert->group partition chosen
to minimize that padding (~1% over the ideal 4096 pairs/group).

Device kernel (per core, fp16 matmuls, fp32 accum):
  mm1: stationary = W1 slice chunk [128k, 128h], moving = X^T slab
       [128k, S] -> H^T chunk [128h, S] PSUM (8 k-chunks); ScalarE
       tanh-gelu (+b1), VectorE multiply by per-pair gate weight (fp16).
  mm2: stationary = W2 slice chunk [128h, 128d], moving = H^T chunk
       -> Y^T [128d, S] PSUM (8 h-chunks), fp16 copy out (split across
       scalar+vector), one slab-blocked DMA out.
Software pipeline: mm2 of slab s is emitted after mm1 of slab s+1 so the
PE never stalls on a slab's last gelu. DMA issue is spread over engine
sequencers (gpsimd: weight stream + out, sync: X, scalar: b1 + gate rows)
and the 16MB weight stream is popped in ~1MB pieces per slab boundary in
strict first-need order.
"""

import os
import sys

import numpy as np

for _p in ("/opt/trn_rl_repo",):
    if _p not in sys.path and os.path.isdir(_p):
        sys.path.insert(0, _p)

N_TOKENS = 4096
D_MODEL = 1024
D_HIDDEN = 4096
N_EXPERT = 8
TOP_K = 2
P = 128
KO = D_MODEL // P  # 8 contraction chunks for mm1
NCORES = 8
NGROUPS = 2
TPK = NCORES // NGROUPS  # 4-way tensor parallel within a group
EPG = N_EXPERT // NGROUPS  # 4 experts per group
HB = D_HIDDEN // TPK  # 1024-wide hidden slice per core
HO = HB // P  # 8 h-chunks per core for mm2 contraction
DM = D_MODEL // P  # 8 output-partition chunks of Y^T
SLAB = 512  # max moving-dim per matmul (one PSUM bank of fp32)
WARMUP_MM = 10

# filled by kernel() for test harness introspection
last_results = None

_nc_cache = {}


def _even_split(L, cap=SLAB):
    """Split L (a multiple of 4) into even parts <= cap, each a multiple of 4."""
    q = L // 4
    n = -(-L // cap)
    base, extra = divmod(q, n)
    return [4 * (base + 1)] * extra + [4 * base] * (n - extra)


def _make_slabs(plan):
    """Slab plan from padded segment lengths: list of (seg_idx, S, col0).
    First slab small (short DMA lead-in), tail tapered (short flush)."""
    slabs = []
    col0 = 0
    nseg = len(plan)
    for i, Lp in enumerate(plan):
        if Lp == 0:
            continue
        if i == 0 and Lp > 256:
            parts = [128] + _even_split(Lp - 128)
        elif i == nseg - 1 and Lp > 768:
            parts = _even_split(Lp - 384) + [256, 128]
        elif i == nseg - 1 and Lp > 256:
            parts = _even_split(Lp - 128) + [128]
        else:
            parts = _even_split(Lp)
        for S in parts:
            slabs.append((i, S, col0))
            col0 += S
    return tuple(slabs), col0


def _group_split(loads):
    """Choose the 4+4 expert partition minimizing the shared (pairwise-max)
    padded plan, and return (groups, plan) with groups' experts sorted by
    descending load."""
    import itertools

    ids = list(range(N_EXPERT))
    best = None
    for combo in itertools.combinations(ids[1:], EPG - 1):
        ga = (0,) + combo
        gb = tuple(i for i in ids if i not in ga)
        la = sorted((loads[e] for e in ga), reverse=True)
        lb = sorted((loads[e] for e in gb), reverse=True)
        plan = tuple(-(-max(a, b) // 4) * 4 for a, b in zip(la, lb))
        cost = sum(plan)
        if best is None or cost < best[0]:
            ea = tuple(sorted(ga, key=lambda e: -loads[e]))
            eb = tuple(sorted(gb, key=lambda e: -loads[e]))
            best = (cost, (ea, eb), plan)
    return best[1], best[2]


def _build_bass(slabs, cols):
    import concourse.mybir as mybir
    from concourse import bacc
    from concourse.tile import TileContext

    f16 = mybir.dt.float16
    f32 = mybir.dt.float32
    GELU = mybir.ActivationFunctionType.Gelu_apprx_tanh

    nc = bacc.Bacc("TRN2", target_bir_lowering=False, debug=False, num_devices=NCORES)

    SMAX = max(S for _, S, _ in slabs)

    x_d = nc.declare_dram_parameter("x", [P, KO * cols], f16, isOutput=False)
    w1_d = nc.declare_dram_parameter("w1", [EPG, P, KO, HB], f16, isOutput=False)
    w2_d = nc.declare_dram_parameter("w2", [EPG, P, HO, D_MODEL], f16, isOutput=False)
    b1_d = nc.declare_dram_parameter("b1", [P, EPG, HO], f32, isOutput=False)
    wb_d = nc.declare_dram_parameter("wb", [P, cols], f16, isOutput=False)
    # slab-blocked output: per slab a contiguous [P, DM*S] block at DM*col0
    out_d = nc.declare_dram_parameter("out", [P, DM * cols], f16, isOutput=True)

    seg_first_slab = {}
    for si, (sg, S, c0) in enumerate(slabs):
        seg_first_slab.setdefault(sg, si)
    segs_used = sorted(seg_first_slab)

    with TileContext(nc) as tc:
        with (
            tc.tile_pool(name="wpool", bufs=1) as wpool,
            tc.tile_pool(name="xpool", bufs=3) as xpool,
            tc.tile_pool(name="wbpool", bufs=3) as wbpool,
            tc.tile_pool(name="hpool", bufs=2) as hpool,
            tc.tile_pool(name="ypool", bufs=2) as ypool,
            tc.tile_pool(name="ps1", bufs=3, space="PSUM") as ps1,
            tc.tile_pool(name="ps2", bufs=4, space="PSUM") as ps2,
        ):
            # PE warm-up: dependency-free matmuls keep PE busy (and HAM
            # warming) through the preamble barrier + DMA lead-in.
            warm = wpool.tile([P, 512], f16)
            nc.vector.memset(warm, 0.0)
            wps = ps1.tile([P, SMAX], mybir.dt.float32, tag="hps")
            for _ in range(WARMUP_MM):
                nc.tensor.matmul(
                    wps[:, :512], lhsT=warm[:, :P], rhs=warm, start=True, stop=True
                )

            b1_sb = wpool.tile([P, EPG, HO], f32)
            w1_sb = wpool.tile([P, EPG, KO, HB], f16)
            w2_sb = wpool.tile([P, EPG, HO, D_MODEL], f16)

            # Weight stream in strict first-need order, ~1MB pieces so the
            # early HBM window (8 cores all loading) isn't oversubscribed.
            wq = []
            s0 = segs_used[0]
            for sg in segs_used:
                if sg == s0:
                    wq.append(("w1", sg, 0, 128))
                    wq.append(("w1", sg, 128, 512))
                    wq.append(("w1", sg, 512, HB))
                else:
                    wq.append(("w1", sg, 0, 512))
                    wq.append(("w1", sg, 512, HB))
                wq.append(("w2", sg, 0, 512))
                wq.append(("w2", sg, 512, D_MODEL))

            def issue_weight():
                if not wq:
                    return
                kind, sg, lo, hi = wq.pop(0)
                if kind == "w1":
                    nc.gpsimd.dma_start(
                        w1_sb[:, sg, :, lo:hi], w1_d[sg][:, :, lo:hi]
                    )
                else:
                    nc.gpsimd.dma_start(
                        w2_sb[:, sg, :, lo:hi], w2_d[sg][:, :, lo:hi]
                    )

            nc.scalar.dma_start(b1_sb, b1_d[:, :, :])
            for _ in range(4):
                issue_weight()

            def mm1_slab(si):
                sg, S, c0 = slabs[si]
                x_sb = xpool.tile([P, KO, SMAX], f16, tag="x", name="x_sb")[:, :, :S]
                x_src = x_d[:, KO * c0 : KO * (c0 + S)].rearrange(
                    "p (ko t) -> p ko t", t=S
                )
                # two dma_starts -> two parallel HW queues
                nc.sync.dma_start(x_sb[:, : KO // 2, :], x_src[:, : KO // 2, :])
                nc.sync.dma_start(x_sb[:, KO // 2 :, :], x_src[:, KO // 2 :, :])
                if si >= 1:
                    issue_weight()
                    issue_weight()
                wb_t = wbpool.tile([P, SMAX], f16, tag="wb", name="wb_t")[:, :S]
                nc.scalar.dma_start(wb_t, wb_d[:, c0 : c0 + S])
                h_sb = hpool.tile([P, HO, SMAX], f16, tag="h", name="h_sb")[:, :, :S]
                for ho in range(HO):
                    hps = ps1.tile(
                        [P, SMAX], mybir.dt.float32, tag="hps", name="hps"
                    )[:, :S]
                    for k in range(KO):
                        nc.tensor.matmul(
                            hps,
                            lhsT=w1_sb[:, sg, k, ho * P : (ho + 1) * P],
                            rhs=x_sb[:, k, :],
                            start=(k == 0),
                            stop=(k == KO - 1),
                        )
                    nc.scalar.activation(
                        h_sb[:, ho, :], hps, GELU, bias=b1_sb[:, sg, ho : ho + 1]
                    )
                    # fold the per-pair gate weight into H (fp16, free dim)
                    nc.vector.tensor_mul(h_sb[:, ho, :], h_sb[:, ho, :], wb_t)
                return h_sb

            def mm2_slab(si, h_sb):
                sg, S, c0 = slabs[si]
                last2 = si >= len(slabs) - 2
                y_all = ypool.tile([P, DM, SMAX], f16, tag="y", name="y_sb")[:, :, :S]
                out_dst = out_d[:, DM * c0 : DM * (c0 + S)].rearrange(
                    "p (m t) -> p m t", t=S
                )
                for m in range(DM):
                    yps = ps2.tile(
                        [P, SMAX], mybir.dt.float32, tag="yps", name="yps"
                    )[:, :S]
                    for ho in range(HO):
                        nc.tensor.matmul(
                            yps,
                            lhsT=w2_sb[:, sg, ho, m * P : (m + 1) * P],
                            rhs=h_sb[:, ho, :],
                            start=(ho == 0),
                            stop=(ho == HO - 1),
                        )
                    # PSUM->SBUF fp16 copy split across both engines
                    h2 = (S // 8) * 4
                    nc.scalar.copy(y_all[:, m, :h2], yps[:, :h2])
                    nc.vector.tensor_copy(y_all[:, m, h2:], yps[:, h2:])
                    # tail slabs: flush halves from the (idle-by-then) sync
                    # engine so the final out-DMA overlaps the last matmuls
                    if last2 and m == DM // 2 - 1:
                        nc.sync.dma_start(out_dst[:, : DM // 2], y_all[:, : DM // 2])
                if last2:
                    nc.sync.dma_start(out_dst[:, DM // 2 :], y_all[:, DM // 2 :])
                else:
                    nc.gpsimd.dma_start(out_dst, y_all)

            # software pipeline: mm1(s+1) before mm2(s)
            h_prev = mm1_slab(0)
            for si in range(1, len(slabs)):
                h_cur = mm1_slab(si)
                mm2_slab(si - 1, h_prev)
                h_prev = h_cur
            mm2_slab(len(slabs) - 1, h_prev)
    nc.compile()
    return nc


def _route(moe_inp, Wg, bg):
    """Host gate: replicates NaiveGate (linear logits, top-2, softmax over the
    selected logits). Returns per-expert (token_idx, combine_weight)."""
    logits = moe_inp.astype(np.float32) @ Wg.astype(np.float32) + bg.astype(np.float32)
    order = np.argsort(-logits, axis=1, kind="stable")  # ties -> lower index first
    top_idx = order[:, :TOP_K]
    top_val = np.take_along_axis(logits, top_idx, axis=1)
    m = top_val.max(axis=1, keepdims=True)
    e = np.exp(top_val - m)
    gate = (e / e.sum(axis=1, keepdims=True)).astype(np.float32)
    toks, weights = [], []
    for ex in range(N_EXPERT):
        mask = top_idx == ex  # [N, K]; each token matches at most one slot
        t = np.nonzero(mask.any(axis=1))[0]
        w = gate[mask]  # row-major -> ascending token order, matches t
        toks.append(t)
        weights.append(w)
    return toks, weights


def kernel(**inputs):
    global last_results
    from concourse.bass_utils import run_bass_kernel_spmd

    moe_inp = np.asarray(inputs["moe_inp"], dtype=np.float32)
    Wg = np.asarray(inputs["Wg"], dtype=np.float32)
    bg = np.asarray(inputs["bg"], dtype=np.float32)
    W1 = np.asarray(inputs["W1"], dtype=np.float32)
    b1 = np.asarray(inputs["b1"], dtype=np.float32)
    W2 = np.asarray(inputs["W2"], dtype=np.float32)
    b2 = np.asarray(inputs["b2"], dtype=np.float32)

    toks, weights = _route(moe_inp, Wg, bg)
    loads = [len(t) for t in toks]
    groups, plan = _group_split(loads)
    slabs, cols = _make_slabs(plan)

    if slabs not in _nc_cache:
        _nc_cache[slabs] = _build_bass(slabs, cols)
    nc = _nc_cache[slabs]

    seg_c0 = {}
    for sg, S, c0 in slabs:
        if sg not in seg_c0:
            seg_c0[sg] = c0

    # per-group dispatched X^T / gate rows (segments padded to the plan)
    garrs = []
    for g in range(NGROUPS):
        xT = np.zeros((D_MODEL, cols), dtype=np.float16)
        wrow = np.zeros((cols,), dtype=np.float16)
        for i, e in enumerate(groups[g]):
            c0, L = seg_c0[i], loads[e]
            xT[:, c0 : c0 + L] = moe_inp[toks[e]].T
            wrow[c0 : c0 + L] = weights[e]
        blocks = []
        for sg, S, c0 in slabs:
            blocks.append(
                xT[:, c0 : c0 + S].reshape(KO, P, S).transpose(1, 0, 2).reshape(P, KO * S)
            )
        x_arr = np.ascontiguousarray(np.concatenate(blocks, axis=1))
        wb_arr = np.ascontiguousarray(np.broadcast_to(wrow, (P, cols)))
        garrs.append((x_arr, wb_arr))

    in_maps = []
    for c in range(NCORES):
        g, s = divmod(c, TPK)
        gex = list(groups[g])
        lo, hi = s * HB, (s + 1) * HB
        w1_arr = np.ascontiguousarray(
            W1[gex][:, :, lo:hi]
            .astype(np.float16)
            .reshape(EPG, KO, P, HB)
            .transpose(0, 2, 1, 3)
        )
        w2_arr = np.ascontiguousarray(
            W2[gex][:, lo:hi, :]
            .astype(np.float16)
            .reshape(EPG, HO, P, D_MODEL)
            .transpose(0, 2, 1, 3)
        )
        b1_arr = np.ascontiguousarray(
            b1[gex][:, lo:hi].reshape(EPG, HO, P).transpose(2, 0, 1)
        )
        in_maps.append(
            {
                "x": garrs[g][0],
                "w1": w1_arr,
                "w2": w2_arr,
                "b1": b1_arr,
                "wb": garrs[g][1],
            }
        )

    last_results = run_bass_kernel_spmd(nc, in_maps, core_ids=list(range(NCORES)))

    # host combine: per group sum the 4 hidden-slice partials, decode the
    # slab-blocked layout, scatter by segment
    out = np.zeros((N_TOKENS, D_MODEL), dtype=np.float32)
    for g in range(NGROUPS):
        raw = np.zeros((P, DM * cols), dtype=np.float32)
        for s in range(TPK):
            raw += last_results.results[g * TPK + s]["out"].astype(np.float32)
        yT = np.empty((D_MODEL, cols), dtype=np.float32)
        for sg, S, c0 in slabs:
            blk = raw[:, DM * c0 : DM * (c0 + S)].reshape(P, DM, S)
            yT[:, c0 : c0 + S] = blk.transpose(1, 0, 2).reshape(D_MODEL, S)
        for i, e in enumerate(groups[g]):
            c0, L = seg_c0[i], loads[e]
            out[toks[e]] += yT[:, c0 : c0 + L].T + weights[e][:, None] * b2[e][None, :]
    return out


if __name__ == "__main__":
    rng = np.random.default_rng(0)
    demo = {
        "moe_inp": rng.standard_normal((N_TOKENS, D_MODEL), dtype=np.float32),
        "attn_weights": rng.random((4, N_TOKENS, N_TOKENS), dtype=np.float32),
        "Wg": rng.standard_normal((D_MODEL, N_EXPERT), dtype=np.float32) / 32,
        "bg": np.zeros((N_EXPERT,), np.float32),
        "W1": rng.standard_normal((N_EXPERT, D_MODEL, D_HIDDEN), dtype=np.float32) / 32,
        "b1": np.zeros((N_EXPERT, D_HIDDEN), np.float32),
        "W2": rng.standard_normal((N_EXPERT, D_HIDDEN, D_MODEL), dtype=np.float32) / 64,
        "b2": np.zeros((N_EXPERT, D_MODEL), np.float32),
    }
    o = kernel(**demo)
    print(o.shape, o.dtype)


# revision 27
# speedup vs baseline: 1.4410x; 1.0085x over previous
"""FMoE forward (NaiveGate top-2, per-expert FFN, score-weighted combine) on 8 trn2 cores.

Strategy: hybrid expert-parallel x tensor-parallel. Cores split into 2
groups of 4; each group owns 4 experts; within a group each core holds a
1024-wide hidden slice of its 4 experts' W1/W2 (16MB resident in SBUF) and
processes ALL of the group's dispatched token-expert pairs against its
slice. Per-core DMA is ~33MB (vs ~49MB for pure 8-way TP), which keeps
the HBM stream comfortably under the per-core budget, and load balance is
near-perfect: the SPMD slab plan uses the elementwise max of the two
groups' (sorted) segment lengths, with the expert->group partition chosen
to minimize that padding (~1% over the ideal 4096 pairs/group).

Device kernel (per core, fp16 matmuls, fp32 accum):
  mm1: stationary = W1 slice chunk [128k, 128h], moving = X^T slab
       [128k, S] -> H^T chunk [128h, S] PSUM (8 k-chunks); ScalarE
       tanh-gelu (+b1), VectorE multiply by per-pair gate weight (fp16).
  mm2: stationary = W2 slice chunk [128h, 128d], moving = H^T chunk
       -> Y^T [128d, S] PSUM (8 h-chunks), fp16 copy out (split across
       scalar+vector), one slab-blocked DMA out.
Software pipeline: mm2 of slab s is emitted after mm1 of slab s+1 so the
PE never stalls on a slab's last gelu. DMA issue is spread over engine
sequencers (gpsimd: weight stream + out, sync: X, scalar: b1 + gate rows)
and the 16MB weight stream is popped in ~1MB pieces per slab boundary in
strict first-need order.
"""

import os
import sys

import numpy as np

for _p in ("/opt/trn_rl_repo",):
    if _p not in sys.path and os.path.isdir(_p):
        sys.path.insert(0, _p)

N_TOKENS = 4096
D_MODEL = 1024
D_HIDDEN = 4096
N_EXPERT = 8
TOP_K = 2
P = 128
KO = D_MODEL // P  # 8 contraction chunks for mm1
NCORES = 8
NGROUPS = 2
TPK = NCORES // NGROUPS  # 4-way tensor parallel within a group
EPG = N_EXPERT // NGROUPS  # 4 experts per group
HB = D_HIDDEN // TPK  # 1024-wide hidden slice per core
HO = HB // P  # 8 h-chunks per core for mm2 contraction
DM = D_MODEL // P  # 8 output-partition chunks of Y^T
SLAB = 512  # max moving-dim per matmul (hard ISA limit, one PSUM bank)
WARMUP_MM = 12

# filled by kernel() for test harness introspection
last_results = None

_nc_cache = {}


def _even_split(L, cap=SLAB):
    """Split L (a multiple of 4) into even parts <= cap, each a multiple of 4."""
    q = L // 4
    n = -(-L // cap)
    base, extra = divmod(q, n)
    return [4 * (base + 1)] * extra + [4 * base] * (n - extra)


def _make_slabs(plan):
    """Slab plan from padded segment lengths: list of (seg_idx, S, col0).
    Big even slabs only -- short matmuls pay exposed LDWEIGHTS (~60-107ns
    per MM), so no small lead-in/taper slabs."""
    slabs = []
    col0 = 0
    for i, Lp in enumerate(plan):
        if Lp == 0:
            continue
        for S in _even_split(Lp):
            slabs.append((i, S, col0))
            col0 += S
    return tuple(slabs), col0


def _group_split(loads):
    """Choose the 4+4 expert partition minimizing the shared (pairwise-max)
    padded plan, and return (groups, plan) with groups' experts sorted by
    descending load."""
    import itertools

    ids = list(range(N_EXPERT))
    best = None
    for combo in itertools.combinations(ids[1:], EPG - 1):
        ga = (0,) + combo
        gb = tuple(i for i in ids if i not in ga)
        la = sorted((loads[e] for e in ga), reverse=True)
        lb = sorted((loads[e] for e in gb), reverse=True)
        plan = tuple(-(-max(a, b) // 4) * 4 for a, b in zip(la, lb))
        cost = sum(plan)
        if best is None or cost < best[0]:
            ea = tuple(sorted(ga, key=lambda e: -loads[e]))
            eb = tuple(sorted(gb, key=lambda e: -loads[e]))
            best = (cost, (ea, eb), plan)
    return best[1], best[2]


def _build_bass(slabs, cols):
    import concourse.mybir as mybir
    from concourse import bacc
    from concourse.tile import TileContext

    f16 = mybir.dt.float16
    f32 = mybir.dt.float32
    GELU = mybir.ActivationFunctionType.Gelu_apprx_tanh

    nc = bacc.Bacc("TRN2", target_bir_lowering=False, debug=False, num_devices=NCORES)

    SMAX = max(S for _, S, _ in slabs)

    x_d = nc.declare_dram_parameter("x", [P, KO * cols], f16, isOutput=False)
    w1_d = nc.declare_dram_parameter("w1", [EPG, P, KO, HB], f16, isOutput=False)
    w2_d = nc.declare_dram_parameter("w2", [EPG, P, HO, D_MODEL], f16, isOutput=False)
    b1_d = nc.declare_dram_parameter("b1", [P, EPG, HO], f32, isOutput=False)
    wb_d = nc.declare_dram_parameter("wb", [P, cols], f16, isOutput=False)
    # slab-blocked output: per slab a contiguous [P, DM*S] block at DM*col0
    out_d = nc.declare_dram_parameter("out", [P, DM * cols], f16, isOutput=True)

    seg_first_slab = {}
    for si, (sg, S, c0) in enumerate(slabs):
        seg_first_slab.setdefault(sg, si)
    segs_used = sorted(seg_first_slab)

    with TileContext(nc) as tc:
        with (
            tc.tile_pool(name="wpool", bufs=1) as wpool,
            tc.tile_pool(name="xpool", bufs=3) as xpool,
            tc.tile_pool(name="wbpool", bufs=3) as wbpool,
            tc.tile_pool(name="hpool", bufs=2) as hpool,
            tc.tile_pool(name="ypool", bufs=2) as ypool,
            tc.tile_pool(name="ps1", bufs=3, space="PSUM") as ps1,
            tc.tile_pool(name="ps2", bufs=4, space="PSUM") as ps2,
        ):
            # PE warm-up: dependency-free matmuls keep PE busy (and HAM
            # warming) through the preamble barrier + DMA lead-in.
            warm = wpool.tile([P, 512], f16)
            nc.vector.memset(warm, 0.0)
            wps = ps1.tile([P, SMAX], mybir.dt.float32, tag="hps")
            for _ in range(WARMUP_MM):
                nc.tensor.matmul(
                    wps[:, :512], lhsT=warm[:, :P], rhs=warm, start=True, stop=True
                )

            b1_sb = wpool.tile([P, EPG, HO], f32)
            w1_sb = wpool.tile([P, EPG, KO, HB], f16)
            w2_sb = wpool.tile([P, EPG, HO, D_MODEL], f16)

            # Weight stream in strict first-need order, ~1MB pieces so the
            # early HBM window (8 cores all loading) isn't oversubscribed.
            wq = []
            s0 = segs_used[0]
            for sg in segs_used:
                if sg == s0:
                    wq.append(("w1", sg, 0, 128))
                    wq.append(("w1", sg, 128, 512))
                    wq.append(("w1", sg, 512, HB))
                else:
                    wq.append(("w1", sg, 0, 512))
                    wq.append(("w1", sg, 512, HB))
                wq.append(("w2", sg, 0, 512))
                wq.append(("w2", sg, 512, D_MODEL))

            def issue_weight():
                if not wq:
                    return
                kind, sg, lo, hi = wq.pop(0)
                if kind == "w1":
                    nc.gpsimd.dma_start(
                        w1_sb[:, sg, :, lo:hi], w1_d[sg][:, :, lo:hi]
                    )
                else:
                    nc.gpsimd.dma_start(
                        w2_sb[:, sg, :, lo:hi], w2_d[sg][:, :, lo:hi]
                    )

            nc.scalar.dma_start(b1_sb, b1_d[:, :, :])
            for _ in range(4):
                issue_weight()

            def mm1_slab(si):
                sg, S, c0 = slabs[si]
                x_sb = xpool.tile([P, KO, SMAX], f16, tag="x", name="x_sb")[:, :, :S]
                x_src = x_d[:, KO * c0 : KO * (c0 + S)].rearrange(
                    "p (ko t) -> p ko t", t=S
                )
                # two dma_starts -> two parallel HW queues
                nc.sync.dma_start(x_sb[:, : KO // 2, :], x_src[:, : KO // 2, :])
                nc.sync.dma_start(x_sb[:, KO // 2 :, :], x_src[:, KO // 2 :, :])
                if si >= 1:
                    issue_weight()
                    issue_weight()
                    if si <= 2:
                        issue_weight()
                wb_t = wbpool.tile([P, SMAX], f16, tag="wb", name="wb_t")[:, :S]
                nc.scalar.dma_start(wb_t, wb_d[:, c0 : c0 + S])
                h_sb = hpool.tile([P, HO, SMAX], f16, tag="h", name="h_sb")[:, :, :S]
                for ho in range(HO):
                    hps = ps1.tile(
                        [P, SMAX], mybir.dt.float32, tag="hps", name="hps"
                    )[:, :S]
                    for k in range(KO):
                        nc.tensor.matmul(
                            hps,
                            lhsT=w1_sb[:, sg, k, ho * P : (ho + 1) * P],
                            rhs=x_sb[:, k, :],
                            start=(k == 0),
                            stop=(k == KO - 1),
                        )
                    nc.scalar.activation(
                        h_sb[:, ho, :], hps, GELU, bias=b1_sb[:, sg, ho : ho + 1]
                    )
                    # fold the per-pair gate weight into H (fp16, free dim)
                    nc.vector.tensor_mul(h_sb[:, ho, :], h_sb[:, ho, :], wb_t)
                return h_sb

            def mm2_slab(si, h_sb):
                sg, S, c0 = slabs[si]
                last2 = si >= len(slabs) - 2
                y_all = ypool.tile([P, DM, SMAX], f16, tag="y", name="y_sb")[:, :, :S]
                out_dst = out_d[:, DM * c0 : DM * (c0 + S)].rearrange(
                    "p (m t) -> p m t", t=S
                )
                for m in range(DM):
                    yps = ps2.tile(
                        [P, SMAX], mybir.dt.float32, tag="yps", name="yps"
                    )[:, :S]
                    for ho in range(HO):
                        nc.tensor.matmul(
                            yps,
                            lhsT=w2_sb[:, sg, ho, m * P : (m + 1) * P],
                            rhs=h_sb[:, ho, :],
                            start=(ho == 0),
                            stop=(ho == HO - 1),
                        )
                    # PSUM->SBUF fp16 copy split across both engines
                    h2 = (S // 8) * 4
                    nc.scalar.copy(y_all[:, m, :h2], yps[:, :h2])
                    nc.vector.tensor_copy(y_all[:, m, h2:], yps[:, h2:])
                    # tail slabs: flush halves from the (idle-by-then) sync
                    # engine so the final out-DMA overlaps the last matmuls
                    if last2 and m == DM // 2 - 1:
                        nc.sync.dma_start(out_dst[:, : DM // 2], y_all[:, : DM // 2])
                if last2:
                    nc.sync.dma_start(out_dst[:, DM // 2 :], y_all[:, DM // 2 :])
                else:
                    nc.gpsimd.dma_start(out_dst, y_all)

            # software pipeline: mm1(s+1) before mm2(s)
            h_prev = mm1_slab(0)
            for si in range(1, len(slabs)):
                h_cur = mm1_slab(si)
                mm2_slab(si - 1, h_prev)
                h_prev = h_cur
            mm2_slab(len(slabs) - 1, h_prev)
    nc.compile()
    return nc


def _route(moe_inp, Wg, bg):
    """Host gate: replicates NaiveGate (linear logits, top-2, softmax over the
    selected logits). Returns per-expert (token_idx, combine_weight)."""
    logits = moe_inp.astype(np.float32) @ Wg.astype(np.float32) + bg.astype(np.float32)
    order = np.argsort(-logits, axis=1, kind="stable")  # ties -> lower index first
    top_idx = order[:, :TOP_K]
    top_val = np.take_along_axis(logits, top_idx, axis=1)
    m = top_val.max(axis=1, keepdims=True)
    e = np.exp(top_val - m)
    gate = (e / e.sum(axis=1, keepdims=True)).astype(np.float32)
    toks, weights = [], []
    for ex in range(N_EXPERT):
        mask = top_idx == ex  # [N, K]; each token matches at most one slot
        t = np.nonzero(mask.any(axis=1))[0]
        w = gate[mask]  # row-major -> ascending token order, matches t
        toks.append(t)
        weights.append(w)
    return toks, weights


def kernel(**inputs):
    global last_results
    from concourse.bass_utils import run_bass_kernel_spmd

    moe_inp = np.asarray(inputs["moe_inp"], dtype=np.float32)
    Wg = np.asarray(inputs["Wg"], dtype=np.float32)
    bg = np.asarray(inputs["bg"], dtype=np.float32)
    W1 = np.asarray(inputs["W1"], dtype=np.float32)
    b1 = np.asarray(inputs["b1"], dtype=np.float32)
    W2 = np.asarray(inputs["W2"], dtype=np.float32)
    b2 = np.asarray(inputs["b2"], dtype=np.float32)

    toks, weights = _route(moe_inp, Wg, bg)
    loads = [len(t) for t in toks]
    groups, plan = _group_split(loads)
    slabs, cols = _make_slabs(plan)

    if slabs not in _nc_cache:
        _nc_cache[slabs] = _build_bass(slabs, cols)
    nc = _nc_cache[slabs]

    seg_c0 = {}
    for sg, S, c0 in slabs:
        if sg not in seg_c0:
            seg_c0[sg] = c0

    # per-group dispatched X^T / gate rows (segments padded to the plan)
    garrs = []
    for g in range(NGROUPS):
        xT = np.zeros((D_MODEL, cols), dtype=np.float16)
        wrow = np.zeros((cols,), dtype=np.float16)
        for i, e in enumerate(groups[g]):
            c0, L = seg_c0[i], loads[e]
            xT[:, c0 : c0 + L] = moe_inp[toks[e]].T
            wrow[c0 : c0 + L] = weights[e]
        blocks = []
        for sg, S, c0 in slabs:
            blocks.append(
                xT[:, c0 : c0 + S].reshape(KO, P, S).transpose(1, 0, 2).reshape(P, KO * S)
            )
        x_arr = np.ascontiguousarray(np.concatenate(blocks, axis=1))
        wb_arr = np.ascontiguousarray(np.broadcast_to(wrow, (P, cols)))
        garrs.append((x_arr, wb_arr))

    in_maps = []
    for c in range(NCORES):
        g, s = divmod(c, TPK)
        gex = list(groups[g])
        lo, hi = s * HB, (s + 1) * HB
        w1_arr = np.ascontiguousarray(
            W1[gex][:, :, lo:hi]
            .astype(np.float16)
            .reshape(EPG, KO, P, HB)
            .transpose(0, 2, 1, 3)
        )
        w2_arr = np.ascontiguousarray(
            W2[gex][:, lo:hi, :]
            .astype(np.float16)
            .reshape(EPG, HO, P, D_MODEL)
            .transpose(0, 2, 1, 3)
        )
        b1_arr = np.ascontiguousarray(
            b1[gex][:, lo:hi].reshape(EPG, HO, P).transpose(2, 0, 1)
        )
        in_maps.append(
            {
                "x": garrs[g][0],
                "w1": w1_arr,
                "w2": w2_arr,
                "b1": b1_arr,
                "wb": garrs[g][1],
            }
        )

    last_results = run_bass_kernel_spmd(nc, in_maps, core_ids=list(range(NCORES)))

    # host combine: per group sum the 4 hidden-slice partials, decode the
    # slab-blocked layout, scatter by segment
    out = np.zeros((N_TOKENS, D_MODEL), dtype=np.float32)
    for g in range(NGROUPS):
        raw = np.zeros((P, DM * cols), dtype=np.float32)
        for s in range(TPK):
            raw += last_results.results[g * TPK + s]["out"].astype(np.float32)
        yT = np.empty((D_MODEL, cols), dtype=np.float32)
        for sg, S, c0 in slabs:
            blk = raw[:, DM * c0 : DM * (c0 + S)].reshape(P, DM, S)
            yT[:, c0 : c0 + S] = blk.transpose(1, 0, 2).reshape(D_MODEL, S)
        for i, e in enumerate(groups[g]):
            c0, L = seg_c0[i], loads[e]
            out[toks[e]] += yT[:, c0 : c0 + L].T + weights[e][:, None] * b2[e][None, :]
    return out


if __name__ == "__main__":
    rng = np.random.default_rng(0)
    demo = {
        "moe_inp": rng.standard_normal((N_TOKENS, D_MODEL), dtype=np.float32),
        "attn_weights": rng.random((4, N_TOKENS, N_TOKENS), dtype=np.float32),
        "Wg": rng.standard_normal((D_MODEL, N_EXPERT), dtype=np.float32) / 32,
        "bg": np.zeros((N_EXPERT,), np.float32),
        "W1": rng.standard_normal((N_EXPERT, D_MODEL, D_HIDDEN), dtype=np.float32) / 32,
        "b1": np.zeros((N_EXPERT, D_HIDDEN), np.float32),
        "W2": rng.standard_normal((N_EXPERT, D_HIDDEN, D_MODEL), dtype=np.float32) / 64,
        "b2": np.zeros((N_EXPERT, D_MODEL), np.float32),
    }
    o = kernel(**demo)
    print(o.shape, o.dtype)


# revision 30
# speedup vs baseline: 1.4505x; 1.0066x over previous
"""FMoE forward (NaiveGate top-2, per-expert FFN, score-weighted combine) on 8 trn2 cores.

Strategy: hybrid expert-parallel x tensor-parallel. Cores split into 2
groups of 4; each group owns 4 experts; within a group each core holds a
1024-wide hidden slice of its 4 experts' W1/W2 (16MB resident in SBUF) and
processes ALL of the group's dispatched token-expert pairs against its
slice. Per-core DMA is ~33MB (vs ~49MB for pure 8-way TP), which keeps
the HBM stream comfortably under the per-core budget, and load balance is
near-perfect: the SPMD slab plan uses the elementwise max of the two
groups' (sorted) segment lengths, with the expert->group partition chosen
to minimize that padding (~1% over the ideal 4096 pairs/group).

Device kernel (per core, fp16 matmuls, fp32 accum):
  mm1: stationary = W1 slice chunk [128k, 128h], moving = X^T slab
       [128k, S] -> H^T chunk [128h, S] PSUM (8 k-chunks); ScalarE
       tanh-gelu (+b1), VectorE multiply by per-pair gate weight (fp16).
  mm2: stationary = W2 slice chunk [128h, 128d], moving = H^T chunk
       -> Y^T [128d, S] PSUM (8 h-chunks), fp16 copy out (split across
       scalar+vector), one slab-blocked DMA out.
Software pipeline: mm2 of slab s is emitted after mm1 of slab s+1 so the
PE never stalls on a slab's last gelu. DMA issue is spread over engine
sequencers (gpsimd: weight stream + out, sync: X, scalar: b1 + gate rows)
and the 16MB weight stream is popped in ~1MB pieces per slab boundary in
strict first-need order.
"""

import os
import sys

import numpy as np

for _p in ("/opt/trn_rl_repo",):
    if _p not in sys.path and os.path.isdir(_p):
        sys.path.insert(0, _p)

N_TOKENS = 4096
D_MODEL = 1024
D_HIDDEN = 4096
N_EXPERT = 8
TOP_K = 2
P = 128
KO = D_MODEL // P  # 8 contraction chunks for mm1
NCORES = 8
NGROUPS = 2
TPK = NCORES // NGROUPS  # 4-way tensor parallel within a group
EPG = N_EXPERT // NGROUPS  # 4 experts per group
HB = D_HIDDEN // TPK  # 1024-wide hidden slice per core
HO = HB // P  # 8 h-chunks per core for mm2 contraction
DM = D_MODEL // P  # 8 output-partition chunks of Y^T
SLAB = 512  # max moving-dim per matmul (hard ISA limit, one PSUM bank)
WARMUP_MM = 12

# filled by kernel() for test harness introspection
last_results = None

_nc_cache = {}


def _even_split(L, cap=SLAB):
    """Split L (a multiple of 4) into even parts <= cap, each a multiple of 4."""
    q = L // 4
    n = -(-L // cap)
    base, extra = divmod(q, n)
    return [4 * (base + 1)] * extra + [4 * base] * (n - extra)


def _make_slabs(plan):
    """Slab plan from padded segment lengths: list of (seg_idx, S, col0).
    Big even slabs only -- short matmuls pay exposed LDWEIGHTS (~60-107ns
    per MM), so no small lead-in/taper slabs."""
    slabs = []
    col0 = 0
    for i, Lp in enumerate(plan):
        if Lp == 0:
            continue
        for S in _even_split(Lp):
            slabs.append((i, S, col0))
            col0 += S
    return tuple(slabs), col0


def _group_split(loads):
    """Choose the 4+4 expert partition minimizing the shared (pairwise-max)
    padded plan, and return (groups, plan) with groups' experts sorted by
    descending load."""
    import itertools

    ids = list(range(N_EXPERT))
    best = None
    for combo in itertools.combinations(ids[1:], EPG - 1):
        ga = (0,) + combo
        gb = tuple(i for i in ids if i not in ga)
        la = sorted((loads[e] for e in ga), reverse=True)
        lb = sorted((loads[e] for e in gb), reverse=True)
        plan = tuple(-(-max(a, b) // 4) * 4 for a, b in zip(la, lb))
        cost = sum(plan)
        if best is None or cost < best[0]:
            ea = tuple(sorted(ga, key=lambda e: -loads[e]))
            eb = tuple(sorted(gb, key=lambda e: -loads[e]))
            best = (cost, (ea, eb), plan)
    return best[1], best[2]


def _build_bass(slabs, cols):
    import concourse.mybir as mybir
    from concourse import bacc
    from concourse.tile import TileContext

    f16 = mybir.dt.float16
    f32 = mybir.dt.float32
    GELU = mybir.ActivationFunctionType.Gelu_apprx_tanh

    nc = bacc.Bacc("TRN2", target_bir_lowering=False, debug=False, num_devices=NCORES)

    SMAX = max(S for _, S, _ in slabs)

    x_d = nc.declare_dram_parameter("x", [P, KO * cols], f16, isOutput=False)
    w1_d = nc.declare_dram_parameter("w1", [EPG, P, KO, HB], f16, isOutput=False)
    w2_d = nc.declare_dram_parameter("w2", [EPG, P, HO, D_MODEL], f16, isOutput=False)
    b1_d = nc.declare_dram_parameter("b1", [P, EPG, HO], f32, isOutput=False)
    wb_d = nc.declare_dram_parameter("wb", [P, cols], f16, isOutput=False)
    # slab-blocked output: per slab a contiguous [P, DM*S] block at DM*col0
    out_d = nc.declare_dram_parameter("out", [P, DM * cols], f16, isOutput=True)

    seg_first_slab = {}
    for si, (sg, S, c0) in enumerate(slabs):
        seg_first_slab.setdefault(sg, si)
    segs_used = sorted(seg_first_slab)

    with TileContext(nc) as tc:
        with (
            tc.tile_pool(name="wpool", bufs=1) as wpool,
            tc.tile_pool(name="xpool", bufs=3) as xpool,
            tc.tile_pool(name="wbpool", bufs=3) as wbpool,
            tc.tile_pool(name="hpool", bufs=2) as hpool,
            tc.tile_pool(name="ypool", bufs=2) as ypool,
            tc.tile_pool(name="ps1", bufs=3, space="PSUM") as ps1,
            tc.tile_pool(name="ps2", bufs=4, space="PSUM") as ps2,
        ):
            # PE warm-up: dependency-free matmuls keep PE busy (and HAM
            # warming) through the preamble barrier + DMA lead-in.
            warm = wpool.tile([P, 512], f16)
            nc.vector.memset(warm, 0.0)
            wps = ps1.tile([P, SMAX], mybir.dt.float32, tag="hps")
            for _ in range(WARMUP_MM):
                nc.tensor.matmul(
                    wps[:, :512], lhsT=warm[:, :P], rhs=warm, start=True, stop=True
                )

            b1_sb = wpool.tile([P, EPG, HO], f32)
            w1_sb = wpool.tile([P, EPG, KO, HB], f16)
            w2_sb = wpool.tile([P, EPG, HO, D_MODEL], f16)

            # Weight stream in strict first-need order, ~1MB pieces so the
            # early HBM window (8 cores all loading) isn't oversubscribed.
            wq = []
            s0 = segs_used[0]
            for sg in segs_used:
                if sg == s0:
                    wq.append(("w1", sg, 0, 128))
                    wq.append(("w1", sg, 128, 512))
                    wq.append(("w1", sg, 512, HB))
                else:
                    wq.append(("w1", sg, 0, 512))
                    wq.append(("w1", sg, 512, HB))
                wq.append(("w2", sg, 0, 512))
                wq.append(("w2", sg, 512, D_MODEL))

            def issue_weight():
                if not wq:
                    return
                kind, sg, lo, hi = wq.pop(0)
                if kind == "w1":
                    nc.gpsimd.dma_start(
                        w1_sb[:, sg, :, lo:hi], w1_d[sg][:, :, lo:hi]
                    )
                else:
                    nc.gpsimd.dma_start(
                        w2_sb[:, sg, :, lo:hi], w2_d[sg][:, :, lo:hi]
                    )

            nc.scalar.dma_start(b1_sb, b1_d[:, :, :])
            # upfront: all of segment 0's weights (~4MB); the rest of the
            # stream is paced behind the per-slab out-DMAs on gpsimd's
            # in-order queue, so the 16MB never floods the early HBM window
            for _ in range(5):
                issue_weight()

            def mm1_slab(si):
                sg, S, c0 = slabs[si]
                x_sb = xpool.tile([P, KO, SMAX], f16, tag="x", name="x_sb")[:, :, :S]
                x_src = x_d[:, KO * c0 : KO * (c0 + S)].rearrange(
                    "p (ko t) -> p ko t", t=S
                )
                # two dma_starts -> two parallel HW queues
                nc.sync.dma_start(x_sb[:, : KO // 2, :], x_src[:, : KO // 2, :])
                nc.sync.dma_start(x_sb[:, KO // 2 :, :], x_src[:, KO // 2 :, :])
                wb_t = wbpool.tile([P, SMAX], f16, tag="wb", name="wb_t")[:, :S]
                nc.scalar.dma_start(wb_t, wb_d[:, c0 : c0 + S])
                h_sb = hpool.tile([P, HO, SMAX], f16, tag="h", name="h_sb")[:, :, :S]
                for ho in range(HO):
                    hps = ps1.tile(
                        [P, SMAX], mybir.dt.float32, tag="hps", name="hps"
                    )[:, :S]
                    for k in range(KO):
                        nc.tensor.matmul(
                            hps,
                            lhsT=w1_sb[:, sg, k, ho * P : (ho + 1) * P],
                            rhs=x_sb[:, k, :],
                            start=(k == 0),
                            stop=(k == KO - 1),
                        )
                    nc.scalar.activation(
                        h_sb[:, ho, :], hps, GELU, bias=b1_sb[:, sg, ho : ho + 1]
                    )
                    # fold the per-pair gate weight into H (fp16, free dim)
                    nc.vector.tensor_mul(h_sb[:, ho, :], h_sb[:, ho, :], wb_t)
                return h_sb

            def mm2_slab(si, h_sb):
                sg, S, c0 = slabs[si]
                last2 = si >= len(slabs) - 2
                y_all = ypool.tile([P, DM, SMAX], f16, tag="y", name="y_sb")[:, :, :S]
                out_dst = out_d[:, DM * c0 : DM * (c0 + S)].rearrange(
                    "p (m t) -> p m t", t=S
                )
                for m in range(DM):
                    yps = ps2.tile(
                        [P, SMAX], mybir.dt.float32, tag="yps", name="yps"
                    )[:, :S]
                    for ho in range(HO):
                        nc.tensor.matmul(
                            yps,
                            lhsT=w2_sb[:, sg, ho, m * P : (m + 1) * P],
                            rhs=h_sb[:, ho, :],
                            start=(ho == 0),
                            stop=(ho == HO - 1),
                        )
                    # PSUM->SBUF fp16 copy split across both engines
                    h2 = (S // 8) * 4
                    nc.scalar.copy(y_all[:, m, :h2], yps[:, :h2])
                    nc.vector.tensor_copy(y_all[:, m, h2:], yps[:, h2:])
                    # tail slabs: flush m-pair quarters from the (idle by
                    # then) sync engine so the out-DMA overlaps the matmuls
                    if last2 and m % 2 == 1:
                        nc.sync.dma_start(
                            out_dst[:, m - 1 : m + 1], y_all[:, m - 1 : m + 1]
                        )
                if not last2:
                    nc.gpsimd.dma_start(out_dst, y_all)
                    # pace the weight stream behind this slab's out-DMA
                    issue_weight()
                    issue_weight()

            # software pipeline: mm1(s+1) before mm2(s)
            h_prev = mm1_slab(0)
            for si in range(1, len(slabs)):
                h_cur = mm1_slab(si)
                mm2_slab(si - 1, h_prev)
                h_prev = h_cur
            mm2_slab(len(slabs) - 1, h_prev)
    nc.compile()
    return nc


def _route(moe_inp, Wg, bg):
    """Host gate: replicates NaiveGate (linear logits, top-2, softmax over the
    selected logits). Returns per-expert (token_idx, combine_weight)."""
    logits = moe_inp.astype(np.float32) @ Wg.astype(np.float32) + bg.astype(np.float32)
    order = np.argsort(-logits, axis=1, kind="stable")  # ties -> lower index first
    top_idx = order[:, :TOP_K]
    top_val = np.take_along_axis(logits, top_idx, axis=1)
    m = top_val.max(axis=1, keepdims=True)
    e = np.exp(top_val - m)
    gate = (e / e.sum(axis=1, keepdims=True)).astype(np.float32)
    toks, weights = [], []
    for ex in range(N_EXPERT):
        mask = top_idx == ex  # [N, K]; each token matches at most one slot
        t = np.nonzero(mask.any(axis=1))[0]
        w = gate[mask]  # row-major -> ascending token order, matches t
        toks.append(t)
        weights.append(w)
    return toks, weights


def kernel(**inputs):
    global last_results
    from concourse.bass_utils import run_bass_kernel_spmd

    moe_inp = np.asarray(inputs["moe_inp"], dtype=np.float32)
    Wg = np.asarray(inputs["Wg"], dtype=np.float32)
    bg = np.asarray(inputs["bg"], dtype=np.float32)
    W1 = np.asarray(inputs["W1"], dtype=np.float32)
    b1 = np.asarray(inputs["b1"], dtype=np.float32)
    W2 = np.asarray(inputs["W2"], dtype=np.float32)
    b2 = np.asarray(inputs["b2"], dtype=np.float32)

    toks, weights = _route(moe_inp, Wg, bg)
    loads = [len(t) for t in toks]
    groups, plan = _group_split(loads)
    slabs, cols = _make_slabs(plan)

    if slabs not in _nc_cache:
        _nc_cache[slabs] = _build_bass(slabs, cols)
    nc = _nc_cache[slabs]

    seg_c0 = {}
    for sg, S, c0 in slabs:
        if sg not in seg_c0:
            seg_c0[sg] = c0

    # per-group dispatched X^T / gate rows (segments padded to the plan)
    garrs = []
    for g in range(NGROUPS):
        xT = np.zeros((D_MODEL, cols), dtype=np.float16)
        wrow = np.zeros((cols,), dtype=np.float16)
        for i, e in enumerate(groups[g]):
            c0, L = seg_c0[i], loads[e]
            xT[:, c0 : c0 + L] = moe_inp[toks[e]].T
            wrow[c0 : c0 + L] = weights[e]
        blocks = []
        for sg, S, c0 in slabs:
            blocks.append(
                xT[:, c0 : c0 + S].reshape(KO, P, S).transpose(1, 0, 2).reshape(P, KO * S)
            )
        x_arr = np.ascontiguousarray(np.concatenate(blocks, axis=1))
        wb_arr = np.ascontiguousarray(np.broadcast_to(wrow, (P, cols)))
        garrs.append((x_arr, wb_arr))

    in_maps = []
    for c in range(NCORES):
        g, s = divmod(c, TPK)
        gex = list(groups[g])
        lo, hi = s * HB, (s + 1) * HB
        w1_arr = np.ascontiguousarray(
            W1[gex][:, :, lo:hi]
            .astype(np.float16)
            .reshape(EPG, KO, P, HB)
            .transpose(0, 2, 1, 3)
        )
        w2_arr = np.ascontiguousarray(
            W2[gex][:, lo:hi, :]
            .astype(np.float16)
            .reshape(EPG, HO, P, D_MODEL)
            .transpose(0, 2, 1, 3)
        )
        b1_arr = np.ascontiguousarray(
            b1[gex][:, lo:hi].reshape(EPG, HO, P).transpose(2, 0, 1)
        )
        in_maps.append(
            {
                "x": garrs[g][0],
                "w1": w1_arr,
                "w2": w2_arr,
                "b1": b1_arr,
                "wb": garrs[g][1],
            }
        )

    last_results = run_bass_kernel_spmd(nc, in_maps, core_ids=list(range(NCORES)))

    # host combine: per group sum the 4 hidden-slice partials, decode the
    # slab-blocked layout, scatter by segment
    out = np.zeros((N_TOKENS, D_MODEL), dtype=np.float32)
    for g in range(NGROUPS):
        raw = np.zeros((P, DM * cols), dtype=np.float32)
        for s in range(TPK):
            raw += last_results.results[g * TPK + s]["out"].astype(np.float32)
        yT = np.empty((D_MODEL, cols), dtype=np.float32)
        for sg, S, c0 in slabs:
            blk = raw[:, DM * c0 : DM * (c0 + S)].reshape(P, DM, S)
            yT[:, c0 : c0 + S] = blk.transpose(1, 0, 2).reshape(D_MODEL, S)
        for i, e in enumerate(groups[g]):
            c0, L = seg_c0[i], loads[e]
            out[toks[e]] += yT[:, c0 : c0 + L].T + weights[e][:, None] * b2[e][None, :]
    return out


if __name__ == "__main__":
    rng = np.random.default_rng(0)
    demo = {
        "moe_inp": rng.standard_normal((N_TOKENS, D_MODEL), dtype=np.float32),
        "attn_weights": rng.random((4, N_TOKENS, N_TOKENS), dtype=np.float32),
        "Wg": rng.standard_normal((D_MODEL, N_EXPERT), dtype=np.float32) / 32,
        "bg": np.zeros((N_EXPERT,), np.float32),
        "W1": rng.standard_normal((N_EXPERT, D_MODEL, D_HIDDEN), dtype=np.float32) / 32,
        "b1": np.zeros((N_EXPERT, D_HIDDEN), np.float32),
        "W2": rng.standard_normal((N_EXPERT, D_HIDDEN, D_MODEL), dtype=np.float32) / 64,
        "b2": np.zeros((N_EXPERT, D_MODEL), np.float32),
    }
    o = kernel(**demo)
    print(o.shape, o.dtype)
